# revision 1
# baseline (speedup 1.0000x reference)
"""Trainium2 Bass kernel for a TimeSformer-style divided space-time attention block.

Sharding: pure data-parallel over B (8 batch elements -> 8 NeuronCores), no
collectives. Each core computes the full block for one batch element.

Layout strategy per core:
  - residual stream token-major [tokens, 768] fp32 in DRAM; matmul activations
    feature-major ("transposed") bf16 in SBUF; LayerNorm computes per-token
    stats token-major then PE-transposes into the feature-major ln^T tiles
  - all weight@activation GEMMs run token-chunk-outer so consumers of a chunk
    start while later chunks still compute
  - attention uses the S^T trick: S^T = matmul(lhsT=K^T, rhs=Q^T); softmax
    denominators ride a ones-column appended to V; no max-subtraction (post-LN
    logits are small); fp32 exp/stats, bf16 matmul inputs
"""

import numpy as np
import ml_dtypes

import concourse.bass as bass
import concourse.mybir as mybir
import concourse.tile as tile
from concourse import bacc

F32 = mybir.dt.float32
BF16 = mybir.dt.bfloat16
AF = mybir.ActivationFunctionType
ALU = mybir.AluOpType
AX = mybir.AxisListType

D = 768
NH = 12
HD = 64
HID = 3072
B = 8
T = 8
HW = 196
N = 1569
NT = 1568
NF = 197
NS = T * NF
SCALE = HD ** -0.5
P = 128
EPS = 1e-5

T_GROUPS = [(g * P, P) for g in range(12)] + [(12 * P, 32)]


def tiles_of(n, step=128):
    return [(i, min(step, n - i)) for i in range(0, n, step)]


def build_program(sim_gelu=False, loop_n=0):
    nc = bacc.Bacc("TRN2", target_bir_lowering=False, debug=False, num_devices=8)

    def din(name, shape):
        return nc.dram_tensor(name, shape, F32, kind="ExternalInput").ap()

    x = din("x", [N, D])
    g1 = din("g1", [D]); b1 = din("b1", [D])
    Wqkv_s = din("Wqkv_s", [D, 3 * D]); Wproj_s = din("Wproj_s", [D, D]); bproj_s = din("bproj_s", [D])
    gt = din("gt", [D]); bt = din("bt", [D])
    Wqkv_t = din("Wqkv_t", [D, 3 * D]); Wproj_t = din("Wproj_t", [D, D]); bproj_t = din("bproj_t", [D])
    Wtfc = din("Wtfc", [D, D]); btfc = din("btfc", [D])
    g2 = din("g2", [D]); b2 = din("b2", [D])
    W1 = din("W1", [D, HID]); b1m = din("b1m", [HID])
    W2 = din("W2", [HID, D]); b2m = din("b2m", [D])
    maskbd = nc.dram_tensor("maskbd", [P, P], BF16, kind="ExternalInput").ap()
    maskbd4 = nc.dram_tensor("maskbd4", [P, 4 * P], BF16, kind="ExternalInput").ap()
    ident_bf_d = nc.dram_tensor("ident_bf", [P, P], BF16, kind="ExternalInput").ap()
    ident_f_d = nc.dram_tensor("ident_f", [P, P], F32, kind="ExternalInput").ap()

    out = nc.dram_tensor("out", [N, D], F32, kind="ExternalOutput").ap()
    xt_d = nc.dram_tensor("xt_i", [NT, D], F32).ap()
    x2_d = nc.dram_tensor("x2_i", [N, D], F32).ap()

    from contextlib import nullcontext

    with tile.TileContext(nc) as tc:
      with tc.tile_pool(name="const", bufs=1) as const:
        # loads needed by the very first LN chain go first (head of HWDGE queue)
        idb = const.tile([P, P], BF16, tag="idb")
        nc.sync.dma_start(out=idb[:], in_=ident_bf_d)
        eps_sb = const.tile([P, 1], F32, tag="eps")
        nc.vector.memset(eps_sb[:], EPS)

        def load_vec(ap, L, tag):
            t = const.tile([P, L // P], F32, tag=tag, name=tag)
            nc.sync.dma_start(out=t[:], in_=ap.rearrange("(a p) -> p a", p=P))
            return t

        gt_sb = load_vec(gt, D, "gt"); bt_sb = load_vec(bt, D, "bt")
        consts = {}

        def load_late_consts():
            consts["mask"] = const.tile([P, P], BF16, tag="mask", name="mask_sb")
            nc.sync.dma_start(out=consts["mask"][:], in_=maskbd)
            consts["mask4"] = const.tile([P, 4 * P], BF16, tag="mask4", name="mask4_sb")
            nc.sync.dma_start(out=consts["mask4"][:], in_=maskbd4)
            consts["idf"] = const.tile([P, P], F32, tag="idf", name="idf")
            nc.sync.dma_start(out=consts["idf"][:], in_=ident_f_d)
            for nm, ap, L in [("g1", g1, D), ("b1", b1, D), ("g2", g2, D),
                              ("b2", b2, D), ("bprt", bproj_t, D), ("btfc", btfc, D),
                              ("bprs", bproj_s, D), ("b2m", b2m, D), ("b1m", b1m, HID)]:
                consts[nm] = load_vec(ap, L, nm)

        def load_wT(ap, K, M, tag, pool):
            ts = []
            for i, (k0, pk) in enumerate(tiles_of(K)):
                t = pool.tile([P, M], BF16, tag=f"{tag}{i}", name=f"{tag}{i}")
                nc.gpsimd.dma_start(out=t[:], in_=ap[k0:k0 + pk, :])
                ts.append(t)
            return ts

        def ln_to_lnT(pool, ps_tr, src_rows_fn, n_tok, g_sb, b_sb, lnT, name,
                      col0=0, cls_src=None):
            """LayerNorm token tiles from DRAM -> feature-major bf16 lnT tiles."""
            for tok0, pt in tiles_of(n_tok):
                x_sb = pool.tile([P, D], F32, tag=f"{name}x", name=f"{name}x", bufs=3)
                if cls_src is not None and tok0 == 0:
                    nc.sync.dma_start(out=x_sb[0:1], in_=cls_src)
                    nc.sync.dma_start(out=x_sb[1:pt], in_=src_rows_fn(1, pt - 1))
                else:
                    nc.sync.dma_start(out=x_sb[:pt], in_=src_rows_fn(tok0, pt))
                s6 = pool.tile([P, 2, 6], F32, tag=f"{name}s6", name=f"{name}s6")
                nc.vector.bn_stats(s6[:pt, 0], x_sb[:pt, 0:384])
                nc.vector.bn_stats(s6[:pt, 1], x_sb[:pt, 384:768])
                s2 = pool.tile([P, 2], F32, tag=f"{name}s2", name=f"{name}s2")
                nc.vector.bn_aggr(s2[:pt], s6[:pt].rearrange("p a c -> p (a c)"))
                std = pool.tile([P, 1], F32, tag=f"{name}sd", name=f"{name}sd")
                nc.scalar.activation(std[:pt], s2[:pt, 1:2], AF.Sqrt, bias=eps_sb[:pt])
                inv = pool.tile([P, 1], F32, tag=f"{name}iv", name=f"{name}iv")
                nc.vector.reciprocal(inv[:pt], std[:pt])
                xh = pool.tile([P, D], BF16, tag=f"{name}xh", name=f"{name}xh", bufs=2)
                nc.vector.tensor_scalar(xh[:pt], x_sb[:pt], s2[:pt, 0:1], inv[:pt],
                                        ALU.subtract, ALU.mult)
                for j in range(6):
                    ps = ps_tr.tile([P, P], BF16, tag="tr", name="trp")
                    nc.tensor.transpose(ps[:, :pt], xh[:pt, j * P:(j + 1) * P],
                                        idb[:pt, :pt])
                    nc.vector.tensor_scalar(lnT[j][:, col0 + tok0:col0 + tok0 + pt],
                                            ps[:, :pt],
                                            g_sb[:, j:j + 1], b_sb[:, j:j + 1],
                                            ALU.mult, ALU.add)

        def warm_chunks(n_tok):
            # small first chunk so the GEMM starts after ONE upstream LN tile
            out, pos = [(0, P)], P
            while pos < n_tok:
                pc = min(512, n_tok - pos)
                out.append((pos, pc))
                pos += pc
            return out

        def mm_wx(wT, rhsT, m_tiles, n_tok, ps_mm, evict, chunk=512, chunks=None):
            """psum[m, tok] = sum_k wT[k][:, m]^T rhs[k][:, tok]; chunk-outer."""
            for ch0, pc in (chunks if chunks is not None else tiles_of(n_tok, chunk)):
                for mi, (m0, pm) in enumerate(m_tiles):
                    ps = ps_mm.tile([P, chunk], F32, tag="mm", name="mmps")
                    for k in range(len(wT)):
                        nc.tensor.matmul(ps[:pm, :pc],
                                         wT[k][:, m0:m0 + pm],
                                         rhsT[k][:, ch0:ch0 + pc],
                                         start=(k == 0), stop=(k == len(wT) - 1))
                    evict(mi, m0, pm, ch0, pc, ps)

        loop_cm = tc.For_i(0, loop_n, 1) if loop_n else nullcontext()
        with loop_cm:
            # =====================================================
            # Stage T: temporal attention (196 sequences of len 8)
            # =====================================================
            with tc.tile_pool(name="t_main", bufs=1) as main, \
                 tc.tile_pool(name="t_work", bufs=3) as work, \
                 tc.tile_pool(name="t_mm", bufs=2, space="PSUM") as ps_mm, \
                 tc.tile_pool(name="t_tr", bufs=2, space="PSUM") as ps_tr, \
                 tc.tile_pool(name="t_st", bufs=2, space="PSUM") as ps_st, \
                 tc.tile_pool(name="t_ov", bufs=2, space="PSUM") as ps_ov:

                lnT = [main.tile([P, NT], BF16, tag=f"lnT{j}", name=f"lnT{j}")
                       for j in range(6)]
                ln_to_lnT(work, ps_tr, lambda t0, pt: x[1 + t0:1 + t0 + pt, :], NT,
                          gt_sb, bt_sb, lnT, "lnt")

                load_late_consts()
                wqkvT = load_wT(Wqkv_t, D, 3 * D, "wqkvt", main)
                wprT = load_wT(Wproj_t, D, D, "wprt", main)
                wtfcT = load_wT(Wtfc, D, D, "wtfc", main)

                qkT = [main.tile([P, NT], BF16, tag=f"qkT{j}", name=f"qkT{j}")
                       for j in range(12)]

                def ev_qk(mi, m0, pm, ch0, pc, ps):
                    nc.scalar.activation(qkT[mi][:pm, ch0:ch0 + pc], ps[:pm, :pc],
                                         AF.Copy)
                mm_wx(wqkvT, lnT, tiles_of(2 * D), NT, ps_mm, ev_qk,
                      chunks=warm_chunks(NT))

                v_t = [main.tile([P, NH, HD + 1], BF16, tag=f"vt{g}", name=f"vt{g}")
                       for g in range(len(T_GROUPS))]
                for g, (t0, pt) in enumerate(T_GROUPS):
                    nc.vector.memset(v_t[g][:pt, :, HD:HD + 1], 1.0)
                    for half in range(2):
                        ps = ps_mm.tile([P, 512], F32, tag="mm", name="vtps")
                        for k in range(6):
                            nc.tensor.matmul(
                                ps[:pt, :384],
                                lnT[k][:, t0:t0 + pt],
                                wqkvT[k][:, 2 * D + 384 * half:2 * D + 384 * (half + 1)],
                                start=(k == 0), stop=(k == 5))
                        nc.scalar.activation(
                            v_t[g][:pt, 6 * half:6 * (half + 1), 0:HD],
                            ps[:pt, :384].rearrange("p (a c) -> p a c", a=6), AF.Copy)

                oT = [main.tile([P, NT], BF16, tag=f"oT{j}", name=f"oT{j}")
                      for j in range(6)]
                for g, (t0, pt) in enumerate(T_GROUPS):
                    o_tm = work.tile([P, D], BF16, tag="otm", name="otm", bufs=3)
                    for h in range(NH):
                        j, r0 = h // 2, 64 * (h % 2)
                        st = ps_st.tile([P, P], F32, tag="st", name="stps")
                        nc.tensor.matmul(st[:pt, :pt],
                                         qkT[6 + j][r0:r0 + HD, t0:t0 + pt],
                                         qkT[j][r0:r0 + HD, t0:t0 + pt],
                                         start=True, stop=True)
                        es = work.tile([P, P], BF16, tag="es", name="es", bufs=4)
                        nc.scalar.activation(es[:pt, :pt], st[:pt, :pt], AF.Exp,
                                             scale=SCALE)
                        nc.gpsimd.tensor_tensor(es[:pt, :pt], es[:pt, :pt],
                                                consts["mask"][:pt, :pt], ALU.mult)
                        ov = ps_ov.tile([P, HD + 1], F32, tag="ov", name="ovps")
                        nc.tensor.matmul(ov[:pt, :], es[:pt, :pt], v_t[g][:pt, h, :],
                                         start=True, stop=True)
                        rec = work.tile([P, 1], F32, tag="rec", name="rec", bufs=4)
                        nc.vector.reciprocal(rec[:pt], ov[:pt, HD:HD + 1])
                        nc.vector.tensor_scalar_mul(o_tm[:pt, HD * h:HD * (h + 1)],
                                                    ov[:pt, 0:HD], rec[:pt])
                    for j in range(6):
                        ps = ps_tr.tile([P, P], BF16, tag="tr", name="otr")
                        nc.tensor.transpose(ps[:, :pt], o_tm[:pt, j * P:(j + 1) * P],
                                            idb[:pt, :pt])
                        nc.vector.tensor_copy(oT[j][:, t0:t0 + pt], ps[:, :pt])

                pT = [main.tile([P, NT], BF16, tag=f"pT{j}", name=f"pT{j}")
                      for j in range(6)]

                def ev_proj(mi, m0, pm, ch0, pc, ps):
                    nc.vector.tensor_scalar(pT[mi][:pm, ch0:ch0 + pc], ps[:pm, :pc],
                                            1.0, consts["bprt"][:pm, mi:mi + 1],
                                            ALU.mult, ALU.add)
                mm_wx(wprT, oT, tiles_of(D), NT, ps_mm, ev_proj)

                rtT = [main.tile([P, NT], BF16, tag=f"lnT{j}", name=f"rtT{j}")
                       for j in range(6)]

                def ev_tfc(mi, m0, pm, ch0, pc, ps):
                    nc.vector.tensor_scalar(rtT[mi][:pm, ch0:ch0 + pc], ps[:pm, :pc],
                                            1.0, consts["btfc"][:pm, mi:mi + 1],
                                            ALU.mult, ALU.add)
                mm_wx(wtfcT, pT, tiles_of(D), NT, ps_mm, ev_tfc)

                # epilogue: xt = x[1:] + rt -> xt_d (token-major)
                for g, (t0, pt) in enumerate(T_GROUPS):
                    xrow = work.tile([P, D], F32, tag="exr", name="exr", bufs=3)
                    nc.sync.dma_start(out=xrow[:pt], in_=x[1 + t0:1 + t0 + pt, :])
                    xt_sb = work.tile([P, D], F32, tag="ext", name="ext", bufs=3)
                    for j in range(6):
                        ps = ps_tr.tile([P, P], BF16, tag="tr", name="etr")
                        nc.tensor.transpose(ps[:pt, :], rtT[j][:, t0:t0 + pt], idb[:, :])
                        nc.vector.tensor_tensor(xt_sb[:pt, j * P:(j + 1) * P],
                                                ps[:pt, :],
                                                xrow[:pt, j * P:(j + 1) * P], ALU.add)
                    nc.sync.dma_start(out=xt_d[t0:t0 + pt, :], in_=xt_sb[:pt])

            # =====================================================
            # Stage S: spatial attention (8 frames of 197 tokens)
            # =====================================================
            with tc.tile_pool(name="s_main", bufs=1) as main, \
                 tc.tile_pool(name="s_work", bufs=3) as work, \
                 tc.tile_pool(name="s_mm", bufs=2, space="PSUM") as ps_mm, \
                 tc.tile_pool(name="s_tr", bufs=2, space="PSUM") as ps_tr, \
                 tc.tile_pool(name="s_st", bufs=2, space="PSUM") as ps_st, \
                 tc.tile_pool(name="s_ov", bufs=2, space="PSUM") as ps_ov:

                # LN reads xt directly through the frame-permuted AP (no xs
                # round trip); frame token 0 is the cls row x[1]
                lnS = [main.tile([P, NS], BF16, tag=f"lnS{j}", name=f"lnS{j}")
                       for j in range(6)]
                xt_ref = xt_d.rearrange("(n t) c -> n t c", t=T)
                for fr in range(T):
                    f0 = fr * NF
                    for (tk0, ptk) in tiles_of(NF):
                        def src_s(tok0, pt, fr=fr, tk0=tk0):
                            j0 = tk0 + tok0
                            return xt_ref[j0 - 1:j0 - 1 + pt, fr:fr + 1, :]
                        ln_to_lnT(work, ps_tr, src_s, ptk, consts["g1"], consts["b1"],
                                  lnS, "lns", col0=f0 + tk0,
                                  cls_src=(x[1:2, :] if tk0 == 0 else None))

                wqkvS = load_wT(Wqkv_s, D, 3 * D, "wqkvs", main)
                wprS = load_wT(Wproj_s, D, D, "wprs", main)

                qkT = [main.tile([P, NS], BF16, tag=f"qkS{j}", name=f"qkS{j}")
                       for j in range(12)]

                def ev_qks(mi, m0, pm, ch0, pc, ps):
                    nc.scalar.activation(qkT[mi][:pm, ch0:ch0 + pc], ps[:pm, :pc],
                                         AF.Copy)
                mm_wx(wqkvS, lnS, tiles_of(2 * D), NS, ps_mm, ev_qks,
                      chunks=warm_chunks(NS))

                v_s = [[main.tile([P, NH, HD + 1], BF16, tag=f"vs{t}_{i}",
                                  name=f"vs{t}_{i}") for i in range(2)]
                       for t in range(T)]
                for t in range(T):
                    f0 = t * NF
                    for i, (k0, pk) in enumerate(tiles_of(NF)):
                        nc.vector.memset(v_s[t][i][:pk, :, HD:HD + 1], 1.0)
                        for half in range(2):
                            ps = ps_mm.tile([P, 512], F32, tag="mm", name="vsps")
                            for k in range(6):
                                nc.tensor.matmul(
                                    ps[:pk, :384],
                                    lnS[k][:, f0 + k0:f0 + k0 + pk],
                                    wqkvS[k][:, 2 * D + 384 * half:2 * D + 384 * (half + 1)],
                                    start=(k == 0), stop=(k == 5))
                            nc.scalar.activation(
                                v_s[t][i][:pk, 6 * half:6 * (half + 1), 0:HD],
                                ps[:pk, :384].rearrange("p (a c) -> p a c", a=6),
                                AF.Copy)

                oT = [main.tile([P, NS], BF16, tag=f"oS{j}", name=f"oS{j}")
                      for j in range(6)]
                for t in range(T):
                    f0 = t * NF
                    for h in range(NH):
                        j, r0 = h // 2, 64 * (h % 2)
                        qs = qkT[j][r0:r0 + HD, f0:f0 + NF]
                        es_list = []
                        for i, (k0, pk) in enumerate(tiles_of(NF)):
                            st = ps_st.tile([P, NF], F32, tag="st", name="stS")
                            nc.tensor.matmul(st[:pk, :NF],
                                             qkT[6 + j][r0:r0 + HD,
                                                        f0 + k0:f0 + k0 + pk],
                                             qs, start=True, stop=True)
                            es = work.tile([P, NF], BF16, tag="esS", name="esS", bufs=4)
                            nc.scalar.activation(es[:pk, :NF], st[:pk, :NF], AF.Exp,
                                                 scale=SCALE)
                            es_list.append((es, k0, pk))
                        ov = ps_ov.tile([HD + 1, NF], F32, tag="ov", name="ovS")
                        for i, (es, k0, pk) in enumerate(es_list):
                            nc.tensor.matmul(ov[:, :NF], v_s[t][i][:pk, h, :],
                                             es[:pk, :NF],
                                             start=(i == 0), stop=(i == len(es_list) - 1))
                        rec = work.tile([1, NF], F32, tag="recS", name="recS", bufs=4)
                        nc.vector.reciprocal(rec[:1, :], ov[HD:HD + 1, :])
                        bc = work.tile([HD, NF], F32, tag="bcS", name="bcS", bufs=4)
                        nc.gpsimd.partition_broadcast(bc[:, :], rec[:1, :])
                        nc.vector.tensor_tensor(oT[j][r0:r0 + HD, f0:f0 + NF],
                                                ov[0:HD, :NF], bc[:, :], ALU.mult)

                rsT = [main.tile([P, NS], BF16, tag=f"lnS{j}", name=f"rsT{j}")
                       for j in range(6)]

                def ev_projs(mi, m0, pm, ch0, pc, ps):
                    nc.vector.tensor_scalar(rsT[mi][:pm, ch0:ch0 + pc], ps[:pm, :pc],
                                            1.0, consts["bprs"][:pm, mi:mi + 1],
                                            ALU.mult, ALU.add)
                mm_wx(wprS, oT, tiles_of(D), NS, ps_mm, ev_projs)

                # cls_out = mean over frames of rs cls cols; x2[0] = x[1] + cls_out
                cls6 = work.tile([P, 6], F32, tag="cls6", name="cls6")
                for j in range(6):
                    nc.vector.tensor_reduce(
                        cls6[:, j:j + 1],
                        rsT[j].rearrange("p (t n) -> p n t", t=T)[:, 0:1, :],
                        AX.X, ALU.add)
                nc.vector.tensor_scalar_mul(cls6[:, :], cls6[:, :], 1.0 / T)
                psc = ps_mm.tile([P, P], F32, tag="mm", name="psc")
                nc.tensor.transpose(psc[:6, :], cls6[:, 0:6], consts["idf"][:, :])
                x1r = work.tile([6, P], F32, tag="x1r", name="x1r")
                nc.sync.dma_start(out=x1r[:, :],
                                  in_=x[1:2, :].rearrange("a (p c) -> (a p) c", c=P))
                cls_tm = work.tile([6, P], F32, tag="clstm", name="clstm")
                nc.vector.tensor_tensor(cls_tm[:, :], psc[:6, :], x1r[:, :], ALU.add)
                nc.sync.dma_start(out=x2_d[0:1, :].rearrange("a (p c) -> (a p) c", c=P),
                                  in_=cls_tm[:, :])

                # epilogue: x2[1+8*hw+t] = xt[8*hw+t] + rs[t, 1+hw]
                xt_re = xt_d.rearrange("(n t) c -> n t c", t=T)
                x2_re = x2_d[1:N, :].rearrange("(n t) c -> n t c", t=T)
                for (q0, pq) in tiles_of(HW):
                    for t in range(T):
                        f0 = t * NF
                        xrow = work.tile([P, D], F32, tag="sxr", name="sxr", bufs=3)
                        nc.sync.dma_start(out=xrow[:pq], in_=xt_re[q0:q0 + pq, t:t + 1, :])
                        x2_sb = work.tile([P, D], F32, tag="sx2", name="sx2", bufs=3)
                        for j in range(6):
                            ps = ps_tr.tile([P, P], BF16, tag="tr", name="str")
                            nc.tensor.transpose(ps[:pq, :],
                                                rsT[j][:, f0 + 1 + q0:f0 + 1 + q0 + pq],
                                                idb[:, :])
                            nc.vector.tensor_tensor(x2_sb[:pq, j * P:(j + 1) * P],
                                                    ps[:pq, :],
                                                    xrow[:pq, j * P:(j + 1) * P],
                                                    ALU.add)
                        nc.sync.dma_start(out=x2_re[q0:q0 + pq, t:t + 1, :],
                                          in_=x2_sb[:pq])

            # =====================================================
            # Stage M: MLP with exact GELU
            # =====================================================
            with tc.tile_pool(name="m_main", bufs=1) as main, \
                 tc.tile_pool(name="m_work", bufs=3) as work, \
                 tc.tile_pool(name="m_g", bufs=2) as gpool, \
                 tc.tile_pool(name="m_mm", bufs=3, space="PSUM") as ps_mm, \
                 tc.tile_pool(name="m_tr", bufs=3, space="PSUM") as ps_tr:

                lnM = [main.tile([P, N], BF16, tag=f"lnM{j}", name=f"lnM{j}")
                       for j in range(6)]
                ln_to_lnT(work, ps_tr, lambda t0, pt: x2_d[t0:t0 + pt, :], N,
                          consts["g2"], consts["b2"], lnM, "lnm")

                w1T = load_wT(W1, D, HID, "w1", main)
                w2T = load_wT(W2, HID, D, "w2", main)

                for c0, pc in warm_chunks(N):
                    g1T = [gpool.tile([P, 512], BF16, tag=f"g1T{m}", name=f"g1T{m}")
                           for m in range(24)]
                    for m in range(24):
                        ps = ps_mm.tile([P, 512], F32, tag="mm", name="f1ps")
                        for k in range(6):
                            nc.tensor.matmul(ps[:, :pc], w1T[k][:, m * P:(m + 1) * P],
                                             lnM[k][:, c0:c0 + pc],
                                             start=(k == 0), stop=(k == 5))
                        if sim_gelu:
                            hb = work.tile([P, 512], F32, tag="hb", name="hb")
                            nc.vector.tensor_scalar(hb[:, :pc], ps[:, :pc], 1.0,
                                                    consts["b1m"][:, m:m + 1],
                                                    ALU.mult, ALU.add)
                            sg = work.tile([P, 512], F32, tag="sg", name="sg")
                            nc.scalar.activation(sg[:, :pc], hb[:, :pc], AF.Sigmoid,
                                                 scale=1.702)
                            nc.vector.tensor_tensor(g1T[m][:, :pc], hb[:, :pc],
                                                    sg[:, :pc], ALU.mult)
                        else:
                            nc.scalar.activation(g1T[m][:, :pc], ps[:, :pc], AF.Gelu,
                                                 bias=consts["b1m"][:, m:m + 1])
                    o2T = []
                    for mi in range(6):
                        ps = ps_mm.tile([P, 512], F32, tag="mm", name="f2ps")
                        for k in range(24):
                            nc.tensor.matmul(ps[:, :pc], w2T[k][:, mi * P:(mi + 1) * P],
                                             g1T[k][:, :pc],
                                             start=(k == 0), stop=(k == 23))
                        o2 = gpool.tile([P, 512], BF16, tag=f"o2{mi}", name=f"o2{mi}")
                        nc.vector.tensor_scalar(o2[:, :pc], ps[:, :pc], 1.0,
                                                consts["b2m"][:, mi:mi + 1],
                                                ALU.mult, ALU.add)
                        o2T.append(o2)
                    for q0, pq in tiles_of(pc):
                        xrow = work.tile([P, D], F32, tag="mxr", name="mxr", bufs=2)
                        nc.sync.dma_start(out=xrow[:pq],
                                          in_=x2_d[c0 + q0:c0 + q0 + pq, :])
                        ot_sb = work.tile([P, D], F32, tag="mot", name="mot", bufs=2)
                        for j in range(6):
                            ps = ps_tr.tile([P, P], BF16, tag="tr", name="mtr")
                            nc.tensor.transpose(ps[:pq, :], o2T[j][:, q0:q0 + pq],
                                                idb[:, :])
                            nc.vector.tensor_tensor(ot_sb[:pq, j * P:(j + 1) * P],
                                                    ps[:pq, :],
                                                    xrow[:pq, j * P:(j + 1) * P],
                                                    ALU.add)
                        nc.sync.dma_start(out=out[c0 + q0:c0 + q0 + pq, :],
                                          in_=ot_sb[:pq])

    nc.compile()
    return nc


_CACHED = {}


def _get_program():
    if "nc" not in _CACHED:
        _CACHED["nc"] = build_program()
    return _CACHED["nc"]


def _host_consts():
    mask = np.kron(np.eye(16, dtype=np.float32), np.ones((8, 8), np.float32))
    ident = np.eye(P, dtype=np.float32)
    return {
        "maskbd": mask.astype(ml_dtypes.bfloat16),
        "maskbd4": np.tile(mask, (1, 4)).astype(ml_dtypes.bfloat16),
        "ident_bf": ident.astype(ml_dtypes.bfloat16),
        "ident_f": ident,
    }


WNAMES = ["g1", "b1", "Wqkv_s", "Wproj_s", "bproj_s", "gt", "bt", "Wqkv_t",
          "Wproj_t", "bproj_t", "Wtfc", "btfc", "g2", "b2", "W1", "b1m",
          "W2", "b2m"]


def make_in_maps(inputs):
    consts = _host_consts()
    x = np.asarray(inputs["x"], np.float32)
    base = {k: np.ascontiguousarray(np.asarray(inputs[k], np.float32)) for k in WNAMES}
    base.update(consts)
    return [dict(base, x=np.ascontiguousarray(x[i])) for i in range(8)]


def kernel(**inputs):
    nc = _get_program()
    in_maps = make_in_maps(inputs)
    core_ids = list(range(8))
    from concourse.bass_utils import run_bass_kernel_spmd
    res = run_bass_kernel_spmd(nc, in_maps, core_ids)
    return np.stack([res.results[i]["out"] for i in core_ids], axis=0)


if __name__ == "__main__":
    build_program()
    print("built ok")



# revision 4
# speedup vs baseline: 1.2032x; 1.2032x over previous
"""Trainium2 Bass kernel for a TimeSformer-style divided space-time attention block.

Sharding: pure data-parallel over B (8 batch elements -> 8 NeuronCores), no
collectives. Each core computes the full block for one batch element.

Layout strategy per core:
  - residual stream token-major [tokens, 768] fp32 in DRAM; matmul activations
    feature-major ("transposed") bf16 in SBUF; LayerNorm computes per-token
    stats token-major then PE-transposes into the feature-major ln^T tiles
  - all weight@activation GEMMs run token-chunk-outer so consumers of a chunk
    start while later chunks still compute
  - attention uses the S^T trick: S^T = matmul(lhsT=K^T, rhs=Q^T); softmax
    denominators ride a ones-column appended to V; no max-subtraction (post-LN
    logits are small); fp32 exp/stats, bf16 matmul inputs
"""

import numpy as np
import ml_dtypes

import concourse.bass as bass
import concourse.mybir as mybir
import concourse.tile as tile
from concourse import bacc

F32 = mybir.dt.float32
BF16 = mybir.dt.bfloat16
AF = mybir.ActivationFunctionType
ALU = mybir.AluOpType
AX = mybir.AxisListType

D = 768
NH = 12
HD = 64
HID = 3072
B = 8
T = 8
HW = 196
N = 1569
NT = 1568
NF = 197
NS = T * NF
SCALE = HD ** -0.5
P = 128
EPS = 1e-5

T_GROUPS = [(g * P, P) for g in range(12)] + [(12 * P, 32)]


def tiles_of(n, step=128):
    return [(i, min(step, n - i)) for i in range(0, n, step)]


def build_program(sim_gelu=False, loop_n=0):
    nc = bacc.Bacc("TRN2", target_bir_lowering=False, debug=False, num_devices=8)

    def din(name, shape, dt=F32):
        return nc.dram_tensor(name, shape, dt, kind="ExternalInput").ap()

    x = din("x", [N, D])
    g1 = din("g1", [D]); b1 = din("b1", [D])
    Wqkv_s = din("Wqkv_s", [D, 3 * D], BF16); Wproj_s = din("Wproj_s", [D, D], BF16); bproj_s = din("bproj_s", [D])
    gt = din("gt", [D]); bt = din("bt", [D])
    Wqkv_t = din("Wqkv_t", [D, 3 * D], BF16); Wproj_t = din("Wproj_t", [D, D], BF16); bproj_t = din("bproj_t", [D])
    Wtfc = din("Wtfc", [D, D], BF16); btfc = din("btfc", [D])
    g2 = din("g2", [D]); b2 = din("b2", [D])
    W1 = din("W1", [D, HID], BF16); b1m = din("b1m", [HID])
    W2 = din("W2", [HID, D], BF16); b2m = din("b2m", [D])
    maskbd = nc.dram_tensor("maskbd", [P, P], BF16, kind="ExternalInput").ap()
    maskbd4 = nc.dram_tensor("maskbd4", [P, 4 * P], BF16, kind="ExternalInput").ap()
    ident_bf_d = nc.dram_tensor("ident_bf", [P, P], BF16, kind="ExternalInput").ap()
    ident_f_d = nc.dram_tensor("ident_f", [P, P], F32, kind="ExternalInput").ap()

    out = nc.dram_tensor("out", [N, D], F32, kind="ExternalOutput").ap()
    xt_d = nc.dram_tensor("xt_i", [NT, D], F32).ap()
    x2_d = nc.dram_tensor("x2_i", [N, D], F32).ap()

    from contextlib import nullcontext

    with tile.TileContext(nc) as tc:
      with tc.tile_pool(name="const", bufs=1) as const:
        # loads needed by the very first LN chain go first (head of HWDGE queue)
        idb = const.tile([P, P], BF16, tag="idb")
        nc.sync.dma_start(out=idb[:], in_=ident_bf_d)
        eps_sb = const.tile([P, 1], F32, tag="eps")
        nc.vector.memset(eps_sb[:], EPS)

        def load_vec(ap, L, tag):
            t = const.tile([P, L // P], F32, tag=tag, name=tag)
            nc.sync.dma_start(out=t[:], in_=ap.rearrange("(a p) -> p a", p=P))
            return t

        gt_sb = load_vec(gt, D, "gt"); bt_sb = load_vec(bt, D, "bt")
        consts = {}

        def load_late_consts():
            consts["mask"] = const.tile([P, P], BF16, tag="mask", name="mask_sb")
            nc.sync.dma_start(out=consts["mask"][:], in_=maskbd)
            consts["mask4"] = const.tile([P, 4 * P], BF16, tag="mask4", name="mask4_sb")
            nc.sync.dma_start(out=consts["mask4"][:], in_=maskbd4)
            consts["idf"] = const.tile([P, P], F32, tag="idf", name="idf")
            nc.sync.dma_start(out=consts["idf"][:], in_=ident_f_d)
            for nm, ap, L in [("g1", g1, D), ("b1", b1, D), ("g2", g2, D),
                              ("b2", b2, D), ("bprt", bproj_t, D), ("btfc", btfc, D),
                              ("bprs", bproj_s, D), ("b2m", b2m, D), ("b1m", b1m, HID)]:
                consts[nm] = load_vec(ap, L, nm)

        def load_wT(ap, K, M, tag, pool):
            ts = []
            for i, (k0, pk) in enumerate(tiles_of(K)):
                t = pool.tile([P, M], BF16, tag=f"{tag}{i}", name=f"{tag}{i}")
                nc.sync.dma_start(out=t[:], in_=ap[k0:k0 + pk, :])
                ts.append(t)
            return ts

        def ln_to_lnT(pool, ps_tr, src_rows_fn, n_tok, g_sb, b_sb, lnT, name,
                      col0=0, cls_src=None):
            """LayerNorm token tiles from DRAM -> feature-major bf16 lnT tiles."""
            for tok0, pt in tiles_of(n_tok):
                x_sb = pool.tile([P, D], F32, tag=f"{name}x", name=f"{name}x", bufs=3)
                if cls_src is not None and tok0 == 0:
                    nc.sync.dma_start(out=x_sb[0:1], in_=cls_src)
                    nc.sync.dma_start(out=x_sb[1:pt], in_=src_rows_fn(1, pt - 1))
                else:
                    nc.sync.dma_start(out=x_sb[:pt], in_=src_rows_fn(tok0, pt))
                s6 = pool.tile([P, 2, 6], F32, tag=f"{name}s6", name=f"{name}s6")
                nc.vector.bn_stats(s6[:pt, 0], x_sb[:pt, 0:384])
                nc.vector.bn_stats(s6[:pt, 1], x_sb[:pt, 384:768])
                s2 = pool.tile([P, 2], F32, tag=f"{name}s2", name=f"{name}s2")
                nc.vector.bn_aggr(s2[:pt], s6[:pt].rearrange("p a c -> p (a c)"))
                std = pool.tile([P, 1], F32, tag=f"{name}sd", name=f"{name}sd")
                nc.scalar.activation(std[:pt], s2[:pt, 1:2], AF.Sqrt, bias=eps_sb[:pt])
                inv = pool.tile([P, 1], F32, tag=f"{name}iv", name=f"{name}iv")
                nc.vector.reciprocal(inv[:pt], std[:pt])
                xh = pool.tile([P, D], BF16, tag=f"{name}xh", name=f"{name}xh", bufs=2)
                nc.vector.tensor_scalar(xh[:pt], x_sb[:pt], s2[:pt, 0:1], inv[:pt],
                                        ALU.subtract, ALU.mult)
                for j in range(6):
                    ps = ps_tr.tile([P, P], BF16, tag="tr", name="trp")
                    nc.tensor.transpose(ps[:, :pt], xh[:pt, j * P:(j + 1) * P],
                                        idb[:pt, :pt])
                    nc.vector.tensor_scalar(lnT[j][:, col0 + tok0:col0 + tok0 + pt],
                                            ps[:, :pt],
                                            g_sb[:, j:j + 1], b_sb[:, j:j + 1],
                                            ALU.mult, ALU.add)

        def warm_chunks(n_tok):
            # small first chunk so the GEMM starts after ONE upstream LN tile
            out, pos = [(0, P)], P
            while pos < n_tok:
                pc = min(512, n_tok - pos)
                out.append((pos, pc))
                pos += pc
            return out

        def mm_wx(wT, rhsT, m_tiles, n_tok, ps_mm, evict, chunk=512, chunks=None):
            """psum[m, tok] = sum_k wT[k][:, m]^T rhs[k][:, tok]; chunk-outer."""
            for ch0, pc in (chunks if chunks is not None else tiles_of(n_tok, chunk)):
                for mi, (m0, pm) in enumerate(m_tiles):
                    ps = ps_mm.tile([P, chunk], F32, tag="mm", name="mmps")
                    for k in range(len(wT)):
                        nc.tensor.matmul(ps[:pm, :pc],
                                         wT[k][:, m0:m0 + pm],
                                         rhsT[k][:, ch0:ch0 + pc],
                                         start=(k == 0), stop=(k == len(wT) - 1))
                    evict(mi, m0, pm, ch0, pc, ps)

        loop_cm = tc.For_i(0, loop_n, 1) if loop_n else nullcontext()
        with loop_cm:
            # =====================================================
            # Stage T: temporal attention (196 sequences of len 8)
            # =====================================================
            with tc.tile_pool(name="t_main", bufs=1) as main, \
                 tc.tile_pool(name="t_work", bufs=3) as work, \
                 tc.tile_pool(name="t_mm", bufs=2, space="PSUM") as ps_mm, \
                 tc.tile_pool(name="t_tr", bufs=2, space="PSUM") as ps_tr, \
                 tc.tile_pool(name="t_st", bufs=2, space="PSUM") as ps_st, \
                 tc.tile_pool(name="t_ov", bufs=2, space="PSUM") as ps_ov:

                lnT = [main.tile([P, NT], BF16, tag=f"lnT{j}", name=f"lnT{j}")
                       for j in range(6)]
                ln_to_lnT(work, ps_tr, lambda t0, pt: x[1 + t0:1 + t0 + pt, :], NT,
                          gt_sb, bt_sb, lnT, "lnt")

                load_late_consts()
                wqkvT = load_wT(Wqkv_t, D, 3 * D, "wqkvt", main)
                wprT = load_wT(Wproj_t, D, D, "wprt", main)
                wtfcT = load_wT(Wtfc, D, D, "wtfc", main)

                qkT = [main.tile([P, NT], BF16, tag=f"qkT{j}", name=f"qkT{j}")
                       for j in range(12)]

                def ev_qk(mi, m0, pm, ch0, pc, ps):
                    nc.scalar.activation(qkT[mi][:pm, ch0:ch0 + pc], ps[:pm, :pc],
                                         AF.Copy)
                mm_wx(wqkvT, lnT, tiles_of(2 * D), NT, ps_mm, ev_qk,
                      chunks=warm_chunks(NT))

                v_t = [main.tile([P, NH, HD + 1], BF16, tag=f"vt{g}", name=f"vt{g}")
                       for g in range(len(T_GROUPS))]
                for g, (t0, pt) in enumerate(T_GROUPS):
                    nc.vector.memset(v_t[g][:pt, :, HD:HD + 1], 1.0)
                    for half in range(2):
                        ps = ps_mm.tile([P, 512], F32, tag="mm", name="vtps")
                        for k in range(6):
                            nc.tensor.matmul(
                                ps[:pt, :384],
                                lnT[k][:, t0:t0 + pt],
                                wqkvT[k][:, 2 * D + 384 * half:2 * D + 384 * (half + 1)],
                                start=(k == 0), stop=(k == 5))
                        nc.scalar.activation(
                            v_t[g][:pt, 6 * half:6 * (half + 1), 0:HD],
                            ps[:pt, :384].rearrange("p (a c) -> p a c", a=6), AF.Copy)

                oT = [main.tile([P, NT], BF16, tag=f"oT{j}", name=f"oT{j}")
                      for j in range(6)]
                for g, (t0, pt) in enumerate(T_GROUPS):
                    o_tm = work.tile([P, D], BF16, tag="otm", name="otm", bufs=3)
                    for h in range(NH):
                        j, r0 = h // 2, 64 * (h % 2)
                        st = ps_st.tile([P, P], F32, tag="st", name="stps")
                        nc.tensor.matmul(st[:pt, :pt],
                                         qkT[6 + j][r0:r0 + HD, t0:t0 + pt],
                                         qkT[j][r0:r0 + HD, t0:t0 + pt],
                                         start=True, stop=True)
                        es = work.tile([P, P], BF16, tag="es", name="es", bufs=4)
                        nc.scalar.activation(es[:pt, :pt], st[:pt, :pt], AF.Exp,
                                             scale=SCALE)
                        nc.gpsimd.tensor_tensor(es[:pt, :pt], es[:pt, :pt],
                                                consts["mask"][:pt, :pt], ALU.mult)
                        ov = ps_ov.tile([P, HD + 1], F32, tag="ov", name="ovps")
                        nc.tensor.matmul(ov[:pt, :], es[:pt, :pt], v_t[g][:pt, h, :],
                                         start=True, stop=True)
                        rec = work.tile([P, 1], F32, tag="rec", name="rec", bufs=4)
                        nc.vector.reciprocal(rec[:pt], ov[:pt, HD:HD + 1])
                        nc.vector.tensor_scalar_mul(o_tm[:pt, HD * h:HD * (h + 1)],
                                                    ov[:pt, 0:HD], rec[:pt])
                    for j in range(6):
                        ps = ps_tr.tile([P, P], BF16, tag="tr", name="otr")
                        nc.tensor.transpose(ps[:, :pt], o_tm[:pt, j * P:(j + 1) * P],
                                            idb[:pt, :pt])
                        nc.vector.tensor_copy(oT[j][:, t0:t0 + pt], ps[:, :pt])

                pT = [main.tile([P, NT], BF16, tag=f"pT{j}", name=f"pT{j}")
                      for j in range(6)]

                def ev_proj(mi, m0, pm, ch0, pc, ps):
                    nc.vector.tensor_scalar(pT[mi][:pm, ch0:ch0 + pc], ps[:pm, :pc],
                                            1.0, consts["bprt"][:pm, mi:mi + 1],
                                            ALU.mult, ALU.add)
                mm_wx(wprT, oT, tiles_of(D), NT, ps_mm, ev_proj)

                rtT = [main.tile([P, NT], BF16, tag=f"lnT{j}", name=f"rtT{j}")
                       for j in range(6)]

                def ev_tfc(mi, m0, pm, ch0, pc, ps):
                    nc.vector.tensor_scalar(rtT[mi][:pm, ch0:ch0 + pc], ps[:pm, :pc],
                                            1.0, consts["btfc"][:pm, mi:mi + 1],
                                            ALU.mult, ALU.add)
                mm_wx(wtfcT, pT, tiles_of(D), NT, ps_mm, ev_tfc)

                # epilogue: xt = x[1:] + rt -> xt_d (token-major)
                for g, (t0, pt) in enumerate(T_GROUPS):
                    xrow = work.tile([P, D], F32, tag="exr", name="exr", bufs=3)
                    nc.sync.dma_start(out=xrow[:pt], in_=x[1 + t0:1 + t0 + pt, :])
                    xt_sb = work.tile([P, D], F32, tag="ext", name="ext", bufs=3)
                    for j in range(6):
                        ps = ps_tr.tile([P, P], BF16, tag="tr", name="etr")
                        nc.tensor.transpose(ps[:pt, :], rtT[j][:, t0:t0 + pt], idb[:, :])
                        nc.vector.tensor_tensor(xt_sb[:pt, j * P:(j + 1) * P],
                                                ps[:pt, :],
                                                xrow[:pt, j * P:(j + 1) * P], ALU.add)
                    nc.sync.dma_start(out=xt_d[t0:t0 + pt, :], in_=xt_sb[:pt])

            # =====================================================
            # Stage S: spatial attention (8 frames of 197 tokens)
            # =====================================================
            with tc.tile_pool(name="s_main", bufs=1) as main, \
                 tc.tile_pool(name="s_work", bufs=3) as work, \
                 tc.tile_pool(name="s_mm", bufs=2, space="PSUM") as ps_mm, \
                 tc.tile_pool(name="s_tr", bufs=2, space="PSUM") as ps_tr, \
                 tc.tile_pool(name="s_st", bufs=2, space="PSUM") as ps_st, \
                 tc.tile_pool(name="s_ov", bufs=2, space="PSUM") as ps_ov:

                # LN reads xt directly through the frame-permuted AP (no xs
                # round trip); frame token 0 is the cls row x[1]
                lnS = [main.tile([P, NS], BF16, tag=f"lnS{j}", name=f"lnS{j}")
                       for j in range(6)]
                xt_ref = xt_d.rearrange("(n t) c -> n t c", t=T)
                for fr in range(T):
                    f0 = fr * NF
                    for (tk0, ptk) in tiles_of(NF):
                        def src_s(tok0, pt, fr=fr, tk0=tk0):
                            j0 = tk0 + tok0
                            return xt_ref[j0 - 1:j0 - 1 + pt, fr:fr + 1, :]
                        ln_to_lnT(work, ps_tr, src_s, ptk, consts["g1"], consts["b1"],
                                  lnS, "lns", col0=f0 + tk0,
                                  cls_src=(x[1:2, :] if tk0 == 0 else None))

                wqkvS = load_wT(Wqkv_s, D, 3 * D, "wqkvs", main)
                wprS = load_wT(Wproj_s, D, D, "wprs", main)

                qkT = [main.tile([P, NS], BF16, tag=f"qkS{j}", name=f"qkS{j}")
                       for j in range(12)]

                def ev_qks(mi, m0, pm, ch0, pc, ps):
                    nc.scalar.activation(qkT[mi][:pm, ch0:ch0 + pc], ps[:pm, :pc],
                                         AF.Copy)
                mm_wx(wqkvS, lnS, tiles_of(2 * D), NS, ps_mm, ev_qks,
                      chunks=warm_chunks(NS))

                v_s = [[main.tile([P, NH, HD + 1], BF16, tag=f"vs{t}_{i}",
                                  name=f"vs{t}_{i}") for i in range(2)]
                       for t in range(T)]
                for t in range(T):
                    f0 = t * NF
                    for i, (k0, pk) in enumerate(tiles_of(NF)):
                        nc.vector.memset(v_s[t][i][:pk, :, HD:HD + 1], 1.0)
                        for half in range(2):
                            ps = ps_mm.tile([P, 512], F32, tag="mm", name="vsps")
                            for k in range(6):
                                nc.tensor.matmul(
                                    ps[:pk, :384],
                                    lnS[k][:, f0 + k0:f0 + k0 + pk],
                                    wqkvS[k][:, 2 * D + 384 * half:2 * D + 384 * (half + 1)],
                                    start=(k == 0), stop=(k == 5))
                            nc.scalar.activation(
                                v_s[t][i][:pk, 6 * half:6 * (half + 1), 0:HD],
                                ps[:pk, :384].rearrange("p (a c) -> p a c", a=6),
                                AF.Copy)

                oT = [main.tile([P, NS], BF16, tag=f"oS{j}", name=f"oS{j}")
                      for j in range(6)]
                for t in range(T):
                    f0 = t * NF
                    for h in range(NH):
                        j, r0 = h // 2, 64 * (h % 2)
                        qs = qkT[j][r0:r0 + HD, f0:f0 + NF]
                        es_list = []
                        for i, (k0, pk) in enumerate(tiles_of(NF)):
                            st = ps_st.tile([P, NF], F32, tag="st", name="stS")
                            nc.tensor.matmul(st[:pk, :NF],
                                             qkT[6 + j][r0:r0 + HD,
                                                        f0 + k0:f0 + k0 + pk],
                                             qs, start=True, stop=True)
                            es = work.tile([P, NF], BF16, tag="esS", name="esS", bufs=4)
                            nc.scalar.activation(es[:pk, :NF], st[:pk, :NF], AF.Exp,
                                                 scale=SCALE)
                            es_list.append((es, k0, pk))
                        ov = ps_ov.tile([HD + 1, NF], F32, tag="ov", name="ovS")
                        for i, (es, k0, pk) in enumerate(es_list):
                            nc.tensor.matmul(ov[:, :NF], v_s[t][i][:pk, h, :],
                                             es[:pk, :NF],
                                             start=(i == 0), stop=(i == len(es_list) - 1))
                        rec = work.tile([1, NF], F32, tag="recS", name="recS", bufs=4)
                        nc.vector.reciprocal(rec[:1, :], ov[HD:HD + 1, :])
                        bc = work.tile([HD, NF], F32, tag="bcS", name="bcS", bufs=4)
                        nc.gpsimd.partition_broadcast(bc[:, :], rec[:1, :])
                        nc.vector.tensor_tensor(oT[j][r0:r0 + HD, f0:f0 + NF],
                                                ov[0:HD, :NF], bc[:, :], ALU.mult)

                rsT = [main.tile([P, NS], BF16, tag=f"lnS{j}", name=f"rsT{j}")
                       for j in range(6)]

                def ev_projs(mi, m0, pm, ch0, pc, ps):
                    nc.vector.tensor_scalar(rsT[mi][:pm, ch0:ch0 + pc], ps[:pm, :pc],
                                            1.0, consts["bprs"][:pm, mi:mi + 1],
                                            ALU.mult, ALU.add)
                mm_wx(wprS, oT, tiles_of(D), NS, ps_mm, ev_projs)

                # cls_out = mean over frames of rs cls cols; x2[0] = x[1] + cls_out
                cls6 = work.tile([P, 6], F32, tag="cls6", name="cls6")
                for j in range(6):
                    nc.vector.tensor_reduce(
                        cls6[:, j:j + 1],
                        rsT[j].rearrange("p (t n) -> p n t", t=T)[:, 0:1, :],
                        AX.X, ALU.add)
                nc.vector.tensor_scalar_mul(cls6[:, :], cls6[:, :], 1.0 / T)
                psc = ps_mm.tile([P, P], F32, tag="mm", name="psc")
                nc.tensor.transpose(psc[:6, :], cls6[:, 0:6], consts["idf"][:, :])
                x1r = work.tile([6, P], F32, tag="x1r", name="x1r")
                nc.sync.dma_start(out=x1r[:, :],
                                  in_=x[1:2, :].rearrange("a (p c) -> (a p) c", c=P))
                cls_tm = work.tile([6, P], F32, tag="clstm", name="clstm")
                nc.vector.tensor_tensor(cls_tm[:, :], psc[:6, :], x1r[:, :], ALU.add)
                nc.sync.dma_start(out=x2_d[0:1, :].rearrange("a (p c) -> (a p) c", c=P),
                                  in_=cls_tm[:, :])

                # epilogue: x2[1+8*hw+t] = xt[8*hw+t] + rs[t, 1+hw]
                xt_re = xt_d.rearrange("(n t) c -> n t c", t=T)
                x2_re = x2_d[1:N, :].rearrange("(n t) c -> n t c", t=T)
                for (q0, pq) in tiles_of(HW):
                    for t in range(T):
                        f0 = t * NF
                        xrow = work.tile([P, D], F32, tag="sxr", name="sxr", bufs=3)
                        nc.sync.dma_start(out=xrow[:pq], in_=xt_re[q0:q0 + pq, t:t + 1, :])
                        x2_sb = work.tile([P, D], F32, tag="sx2", name="sx2", bufs=3)
                        for j in range(6):
                            ps = ps_tr.tile([P, P], BF16, tag="tr", name="str")
                            nc.tensor.transpose(ps[:pq, :],
                                                rsT[j][:, f0 + 1 + q0:f0 + 1 + q0 + pq],
                                                idb[:, :])
                            nc.vector.tensor_tensor(x2_sb[:pq, j * P:(j + 1) * P],
                                                    ps[:pq, :],
                                                    xrow[:pq, j * P:(j + 1) * P],
                                                    ALU.add)
                        nc.sync.dma_start(out=x2_re[q0:q0 + pq, t:t + 1, :],
                                          in_=x2_sb[:pq])

            # =====================================================
            # Stage M: MLP with exact GELU
            # =====================================================
            with tc.tile_pool(name="m_main", bufs=1) as main, \
                 tc.tile_pool(name="m_work", bufs=3) as work, \
                 tc.tile_pool(name="m_g", bufs=2) as gpool, \
                 tc.tile_pool(name="m_mm", bufs=3, space="PSUM") as ps_mm, \
                 tc.tile_pool(name="m_tr", bufs=3, space="PSUM") as ps_tr:

                lnM = [main.tile([P, N], BF16, tag=f"lnM{j}", name=f"lnM{j}")
                       for j in range(6)]
                ln_to_lnT(work, ps_tr, lambda t0, pt: x2_d[t0:t0 + pt, :], N,
                          consts["g2"], consts["b2"], lnM, "lnm")

                w1T = load_wT(W1, D, HID, "w1", main)
                w2T = load_wT(W2, HID, D, "w2", main)

                for c0, pc in warm_chunks(N):
                    g1T = [gpool.tile([P, 512], BF16, tag=f"g1T{m}", name=f"g1T{m}")
                           for m in range(24)]
                    for m in range(24):
                        ps = ps_mm.tile([P, 512], F32, tag="mm", name="f1ps")
                        for k in range(6):
                            nc.tensor.matmul(ps[:, :pc], w1T[k][:, m * P:(m + 1) * P],
                                             lnM[k][:, c0:c0 + pc],
                                             start=(k == 0), stop=(k == 5))
                        if sim_gelu:
                            hb = work.tile([P, 512], F32, tag="hb", name="hb")
                            nc.vector.tensor_scalar(hb[:, :pc], ps[:, :pc], 1.0,
                                                    consts["b1m"][:, m:m + 1],
                                                    ALU.mult, ALU.add)
                            sg = work.tile([P, 512], F32, tag="sg", name="sg")
                            nc.scalar.activation(sg[:, :pc], hb[:, :pc], AF.Sigmoid,
                                                 scale=1.702)
                            nc.vector.tensor_tensor(g1T[m][:, :pc], hb[:, :pc],
                                                    sg[:, :pc], ALU.mult)
                        else:
                            nc.scalar.activation(g1T[m][:, :pc], ps[:, :pc], AF.Gelu,
                                                 bias=consts["b1m"][:, m:m + 1])
                    o2T = []
                    for mi in range(6):
                        ps = ps_mm.tile([P, 512], F32, tag="mm", name="f2ps")
                        for k in range(24):
                            nc.tensor.matmul(ps[:, :pc], w2T[k][:, mi * P:(mi + 1) * P],
                                             g1T[k][:, :pc],
                                             start=(k == 0), stop=(k == 23))
                        o2 = gpool.tile([P, 512], BF16, tag=f"o2{mi}", name=f"o2{mi}")
                        nc.vector.tensor_scalar(o2[:, :pc], ps[:, :pc], 1.0,
                                                consts["b2m"][:, mi:mi + 1],
                                                ALU.mult, ALU.add)
                        o2T.append(o2)
                    for q0, pq in tiles_of(pc):
                        xrow = work.tile([P, D], F32, tag="mxr", name="mxr", bufs=2)
                        nc.sync.dma_start(out=xrow[:pq],
                                          in_=x2_d[c0 + q0:c0 + q0 + pq, :])
                        ot_sb = work.tile([P, D], F32, tag="mot", name="mot", bufs=2)
                        for j in range(6):
                            ps = ps_tr.tile([P, P], BF16, tag="tr", name="mtr")
                            nc.tensor.transpose(ps[:pq, :], o2T[j][:, q0:q0 + pq],
                                                idb[:, :])
                            nc.vector.tensor_tensor(ot_sb[:pq, j * P:(j + 1) * P],
                                                    ps[:pq, :],
                                                    xrow[:pq, j * P:(j + 1) * P],
                                                    ALU.add)
                        nc.sync.dma_start(out=out[c0 + q0:c0 + q0 + pq, :],
                                          in_=ot_sb[:pq])

    nc.compile()
    return nc


_CACHED = {}


def _get_program():
    if "nc" not in _CACHED:
        _CACHED["nc"] = build_program()
    return _CACHED["nc"]


def _host_consts():
    mask = np.kron(np.eye(16, dtype=np.float32), np.ones((8, 8), np.float32))
    ident = np.eye(P, dtype=np.float32)
    return {
        "maskbd": mask.astype(ml_dtypes.bfloat16),
        "maskbd4": np.tile(mask, (1, 4)).astype(ml_dtypes.bfloat16),
        "ident_bf": ident.astype(ml_dtypes.bfloat16),
        "ident_f": ident,
    }


WNAMES = ["g1", "b1", "Wqkv_s", "Wproj_s", "bproj_s", "gt", "bt", "Wqkv_t",
          "Wproj_t", "bproj_t", "Wtfc", "btfc", "g2", "b2", "W1", "b1m",
          "W2", "b2m"]
BF16_W = {"Wqkv_s", "Wproj_s", "Wqkv_t", "Wproj_t", "Wtfc", "W1", "W2"}


def make_in_maps(inputs):
    consts = _host_consts()
    x = np.asarray(inputs["x"], np.float32)
    base = {}
    for k in WNAMES:
        a = np.asarray(inputs[k], np.float32)
        if k in BF16_W:
            a = a.astype(ml_dtypes.bfloat16)
        base[k] = np.ascontiguousarray(a)
    base.update(consts)
    return [dict(base, x=np.ascontiguousarray(x[i])) for i in range(8)]


def kernel(**inputs):
    nc = _get_program()
    in_maps = make_in_maps(inputs)
    core_ids = list(range(8))
    from concourse.bass_utils import run_bass_kernel_spmd
    res = run_bass_kernel_spmd(nc, in_maps, core_ids)
    return np.stack([res.results[i]["out"] for i in core_ids], axis=0)


if __name__ == "__main__":
    build_program()
    print("built ok")



# revision 5
# speedup vs baseline: 1.2151x; 1.0099x over previous
"""Trainium2 Bass kernel v2 for the TimeSformer-style divided space-time block.

Data-parallel over B (8 cores). Per core, the residual stream lives in SBUF
feature-major as bf16 for the whole block:

  - x loaded once via DMA-transpose (host pre-casts x to bf16, pads to 1664
    rows); no DRAM round trips between the three stages.
  - LayerNorm is computed feature-major: Sum(x) / Sum(x^2) via ones-matmuls on
    the PE (bf16), ACT Square for x^2, then a 2-pass DVE normalize with
    per-token scale/offset rows partition-broadcast by GpSimd.
  - LN's gamma is folded into the following weight matrix on the host;
    LN's beta contributes b@W which is applied as a per-feature bias at
    eviction (q,k) or through the V columns (softmax rows sum to 1).
  - Branch outputs are accumulated into the residual directly from PSUM with
    fused scalar_tensor_tensor evictions (one rounding per residual add).
  - Spatial attention runs on a frame-major copy of the normalized stream
    (strided per-frame normalize); temporal attention runs in token order
    with the S^T block-diagonal mask trick.
"""

import numpy as np
import ml_dtypes

import concourse.bass as bass
import concourse.mybir as mybir
import concourse.tile as tile
from concourse import bacc

F32 = mybir.dt.float32
BF16 = mybir.dt.bfloat16
AF = mybir.ActivationFunctionType
ALU = mybir.AluOpType
AX = mybir.AxisListType

D = 768
KT = 6
NH = 12
HD = 64
HID = 3072
B = 8
T = 8
HWn = 196
N = 1569
NPAD = 1664
NT = 1568
NF = 197
NS = T * NF
SCALE = HD ** -0.5
P = 128
EPS = 1e-5
INV_D = 1.0 / D

CH_T = [(0, 128), (128, 512), (640, 512), (1152, 416)]
CH_M = [(0, 128), (128, 512), (640, 512), (1152, 417)]
CH_QS = [(0, 1), (1, 1), (2, 2), (4, 2), (6, 2)]

BC_QKT = 0
BC_PRT = 12
BC_TFC = 18
BC_QKS = 24
BC_PRS = 36
BC_FC1 = 42
BC_FC2 = 66

VC_MASK = 0
VC_ID = 128
VC_VBT = 256
VC_VBS = 1024


def tiles_of(n, step=128):
    return [(i, min(step, n - i)) for i in range(0, n, step)]


def build_program(loop_n=0, sim_gelu=False):
    nc = bacc.Bacc("TRN2", target_bir_lowering=False, debug=False, num_devices=8)

    xbf = nc.dram_tensor("xbf", [NPAD, D], BF16, kind="ExternalInput").ap()
    wqkv_t_d = nc.dram_tensor("wqkv_t", [D, 3 * D], BF16, kind="ExternalInput").ap()
    wpr_t_d = nc.dram_tensor("wpr_t", [D, D], BF16, kind="ExternalInput").ap()
    wtfc_d = nc.dram_tensor("wtfc", [D, D], BF16, kind="ExternalInput").ap()
    wqkv_s_d = nc.dram_tensor("wqkv_s", [D, 3 * D], BF16, kind="ExternalInput").ap()
    wpr_s_d = nc.dram_tensor("wpr_s", [D, D], BF16, kind="ExternalInput").ap()
    w1_d = nc.dram_tensor("w1", [D, HID], BF16, kind="ExternalInput").ap()
    w2_d = nc.dram_tensor("w2", [HID, D], BF16, kind="ExternalInput").ap()
    biases_d = nc.dram_tensor("biases", [P, 72], F32, kind="ExternalInput").ap()
    vconst_d = nc.dram_tensor("vconst", [P, 1792], BF16, kind="ExternalInput").ap()
    out = nc.dram_tensor("out", [N, D], F32, kind="ExternalOutput").ap()

    from contextlib import nullcontext

    with tile.TileContext(nc) as tc:
      with tc.tile_pool(name="const", bufs=1) as const:
        eps_sb = const.tile([P, 1], F32, tag="eps")
        nc.vector.memset(eps_sb[:], EPS)
        ones_sb = const.tile([P, 8], BF16, tag="ones")
        nc.vector.memset(ones_sb[:], 1.0)

        loop_cm = tc.For_i(0, loop_n, 1) if loop_n else nullcontext()
        with loop_cm:
          with tc.tile_pool(name="glob", bufs=1) as glob:
            xT = [glob.tile([P, NPAD], BF16, tag=f"xT{k}", name=f"xT{k}")
                  for k in range(KT)]
            for k in range(KT):
                nc.sync.dma_start(out=xT[k][:], in_=xbf[:, k * P:(k + 1) * P],
                                  transpose=True)
            Bt = glob.tile([P, 72], F32, tag="biases", name="biases")
            nc.sync.dma_start(out=Bt[:], in_=biases_d)
            vc = glob.tile([P, 1792], BF16, tag="vconst", name="vconst")
            nc.sync.dma_start(out=vc[:], in_=vconst_d)
            mask = vc[:, VC_MASK:VC_MASK + P]
            idb = vc[:, VC_ID:VC_ID + P]

            wq = [glob.tile([P, 3 * D], BF16, tag=f"wq{k}", name=f"wq{k}")
                  for k in range(KT)]
            wp = [glob.tile([P, D], BF16, tag=f"wp{k}", name=f"wp{k}")
                  for k in range(KT)]
            for k in range(KT):
                nc.sync.dma_start(out=wq[k][:], in_=wqkv_t_d[k * P:(k + 1) * P, :])
            for k in range(KT):
                nc.sync.dma_start(out=wp[k][:], in_=wpr_t_d[k * P:(k + 1) * P, :])

            cls_save = glob.tile([P, KT], F32, tag="cls", name="cls_save")
            for k in range(KT):
                nc.vector.tensor_copy(cls_save[:, k:k + 1], xT[k][:, 1:2])

            def ln_stats(pool, ps_pool, src_col0, c0, pc, a_bc, c_bc):
                """Per-token scale/offset rows for xT cols [src_col0+c0, +pc)."""
                psA = ps_pool.tile([P, 512], F32, tag="mm", name="sx")
                for k in range(KT):
                    src = xT[k][:, src_col0 + c0:src_col0 + c0 + pc]
                    nc.tensor.matmul(psA[0:1, :pc], ones_sb[:, 0:1], src,
                                     start=(k == 0), stop=(k == KT - 1))
                psB = ps_pool.tile([P, 512], F32, tag="mm", name="sq")
                for k in range(KT):
                    src = xT[k][:, src_col0 + c0:src_col0 + c0 + pc]
                    sq = pool.tile([P, 512], BF16, tag="sqv", name="sqv", bufs=3)
                    nc.scalar.activation(sq[:, :pc], src, AF.Square)
                    nc.tensor.matmul(psB[0:1, :pc], ones_sb[:, 0:1], sq[:, :pc],
                                     start=(k == 0), stop=(k == KT - 1))
                mu = pool.tile([1, 512], F32, tag="mu", name="mu", bufs=2)
                nc.vector.tensor_scalar_mul(mu[:, :pc], psA[0:1, :pc], INV_D)
                r2 = pool.tile([1, 512], F32, tag="r2", name="r2", bufs=2)
                # r2 = -mu^2 (sign matters: var = E[x^2] - mu^2)
                nc.vector.scalar_tensor_tensor(r2[:, :pc], mu[:, :pc], -1.0,
                                               mu[:, :pc], ALU.mult, ALU.mult)
                nc.vector.scalar_tensor_tensor(r2[:, :pc], psB[0:1, :pc], INV_D,
                                               r2[:, :pc], ALU.mult, ALU.add)
                # inv = exp(-0.5*ln(var+eps)); Ln/Exp share an act table with
                # Square/Identity so no act-table reloads are triggered
                nc.scalar.activation(r2[:, :pc], r2[:, :pc], AF.Ln,
                                     bias=eps_sb[0:1])
                nc.scalar.activation(r2[:, :pc], r2[:, :pc], AF.Exp, scale=-0.5)
                a_row = pool.tile([1, 512], BF16, tag="arow", name="arow", bufs=2)
                nc.vector.tensor_copy(a_row[:, :pc], r2[:, :pc])
                c_row = pool.tile([1, 512], BF16, tag="crow", name="crow", bufs=2)
                nc.vector.tensor_tensor(c_row[:, :pc], mu[:, :pc], r2[:, :pc],
                                        ALU.mult)
                nc.gpsimd.partition_broadcast(a_bc[:, c0:c0 + pc], a_row[0:1, :pc])
                nc.gpsimd.partition_broadcast(c_bc[:, c0:c0 + pc], c_row[0:1, :pc])

            # =====================================================
            # Stage T
            # =====================================================
            with tc.tile_pool(name="t_sb", bufs=1) as sbT, \
                 tc.tile_pool(name="t_work", bufs=3) as work, \
                 tc.tile_pool(name="t_mm", bufs=3, space="PSUM") as pmm, \
                 tc.tile_pool(name="t_att", bufs=3, space="PSUM") as patt, \
                 tc.tile_pool(name="t_tr", bufs=2, space="PSUM") as ptr:

                wtfc = [sbT.tile([P, D], BF16, tag=f"wt{k}", name=f"wt{k}")
                        for k in range(KT)]
                for k in range(KT):
                    nc.sync.dma_start(out=wtfc[k][:], in_=wtfc_d[k * P:(k + 1) * P, :])

                aT = sbT.tile([P, NT], BF16, tag="aT", name="aT")
                cT = sbT.tile([P, NT], BF16, tag="cT", name="cT")
                lnT = [sbT.tile([P, NT], BF16, tag=f"ln{k}", name=f"lnT{k}")
                       for k in range(KT)]
                qkT = [sbT.tile([P, NT], BF16, tag=f"qk{j}", name=f"qkT{j}")
                       for j in range(12)]
                v_t = [sbT.tile([P, NH, HD + 1], BF16, tag=f"vt{g}", name=f"vt{g}")
                       for g in range(13)]
                oT = [sbT.tile([P, NT], BF16, tag=f"oT{k}", name=f"oTt{k}")
                      for k in range(KT)]

                for (c0, pc) in CH_T:
                    ln_stats(work, pmm, 1, c0, pc, aT, cT)
                    for k in range(KT):
                        tmp = work.tile([P, 512], BF16, tag="nt", name="nt", bufs=3)
                        nc.vector.tensor_tensor(tmp[:, :pc],
                                                xT[k][:, 1 + c0:1 + c0 + pc],
                                                aT[:, c0:c0 + pc], ALU.mult)
                        nc.vector.tensor_tensor(lnT[k][:, c0:c0 + pc], tmp[:, :pc],
                                                cT[:, c0:c0 + pc], ALU.subtract)
                    for mi in range(12):
                        ps = pmm.tile([P, 512], F32, tag="mm", name="mm")
                        for k in range(KT):
                            nc.tensor.matmul(ps[:, :pc],
                                             wq[k][:, mi * P:(mi + 1) * P],
                                             lnT[k][:, c0:c0 + pc],
                                             start=(k == 0), stop=(k == KT - 1))
                        nc.scalar.activation(qkT[mi][:, c0:c0 + pc], ps[:, :pc],
                                             AF.Identity, bias=Bt[:, BC_QKT + mi:BC_QKT + mi + 1])
                    for (g0, gp) in tiles_of(pc):
                        g = (c0 + g0) // P
                        t0 = c0 + g0
                        nc.vector.memset(v_t[g][:gp, :, HD:HD + 1], 1.0)
                        for half in range(2):
                            ps = pmm.tile([P, 512], F32, tag="mm", name="mmv")
                            for k in range(KT):
                                nc.tensor.matmul(
                                    ps[:gp, :384],
                                    lnT[k][:, t0:t0 + gp],
                                    wq[k][:, 2 * D + 384 * half:2 * D + 384 * (half + 1)],
                                    start=(k == 0), stop=(k == KT - 1))
                            nc.vector.scalar_tensor_tensor(
                                v_t[g][:gp, 6 * half:6 * (half + 1), 0:HD],
                                ps[:gp, :384].rearrange("p (a c) -> p a c", a=6),
                                1.0,
                                vc[0:gp, VC_VBT + 384 * half:VC_VBT + 384 * (half + 1)]
                                .rearrange("p (a c) -> p a c", a=6),
                                ALU.mult, ALU.add)
                    for (g0, gp) in tiles_of(pc):
                        g = (c0 + g0) // P
                        t0 = c0 + g0
                        o_tm = work.tile([P, D], BF16, tag="otm", name="otm", bufs=2)
                        for h in range(NH):
                            j, r0 = h // 2, HD * (h % 2)
                            att = patt.tile([P, P + HD + 1], F32, tag="att",
                                            name="att")
                            st = att[:, 0:P]
                            ov = att[:, P:P + HD + 1]
                            nc.tensor.matmul(st[:gp, :gp],
                                             qkT[6 + j][r0:r0 + HD, t0:t0 + gp],
                                             qkT[j][r0:r0 + HD, t0:t0 + gp],
                                             start=True, stop=True)
                            es = work.tile([P, P], BF16, tag="es", name="es", bufs=4)
                            nc.scalar.activation(es[:gp, :gp], st[:gp, :gp], AF.Exp,
                                                 scale=SCALE)
                            nc.gpsimd.tensor_tensor(es[:gp, :gp], es[:gp, :gp],
                                                    mask[0:gp, 0:gp], ALU.mult)
                            nc.tensor.matmul(ov[:gp, :], es[:gp, :gp],
                                             v_t[g][:gp, h, :], start=True, stop=True)
                            rec = work.tile([P, 1], F32, tag="rec", name="rec", bufs=4)
                            nc.vector.reciprocal(rec[:gp], ov[:gp, HD:HD + 1])
                            nc.vector.tensor_scalar_mul(o_tm[:gp, HD * h:HD * (h + 1)],
                                                        ov[:gp, 0:HD], rec[:gp])
                        for k in range(KT):
                            ps = ptr.tile([P, P], BF16, tag="tr", name="tr")
                            nc.tensor.transpose(ps[:, :gp], o_tm[:gp, k * P:(k + 1) * P],
                                                idb[0:gp, 0:gp])
                            nc.vector.tensor_copy(oT[k][:, t0:t0 + gp], ps[:, :gp])

                # proj -> pT (chunk-local) -> tfc -> residual accumulate
                for (c0, pc) in CH_T:
                    pTc = []
                    for mi in range(KT):
                        ps = pmm.tile([P, 512], F32, tag="mm", name="mmp")
                        for k in range(KT):
                            nc.tensor.matmul(ps[:, :pc],
                                             wp[k][:, mi * P:(mi + 1) * P],
                                             oT[k][:, c0:c0 + pc],
                                             start=(k == 0), stop=(k == KT - 1))
                        pT = work.tile([P, 512], BF16, tag=f"pT{mi}",
                                       name=f"pT{mi}", bufs=2)
                        nc.scalar.activation(pT[:, :pc], ps[:, :pc],
                                             AF.Identity, bias=Bt[:, BC_PRT + mi:BC_PRT + mi + 1])
                        pTc.append(pT)
                    for mi in range(KT):
                        ps = pmm.tile([P, 512], F32, tag="mm", name="mmt")
                        for k in range(KT):
                            nc.tensor.matmul(ps[:, :pc],
                                             wtfc[k][:, mi * P:(mi + 1) * P],
                                             pTc[k][:, :pc],
                                             start=(k == 0), stop=(k == KT - 1))
                        nc.vector.scalar_tensor_tensor(
                            xT[mi][:, 1 + c0:1 + c0 + pc], ps[:, :pc],
                            Bt[:, BC_TFC + mi:BC_TFC + mi + 1],
                            xT[mi][:, 1 + c0:1 + c0 + pc], ALU.add, ALU.add)

                # spatial weights go into the same tiles, after the last
                # temporal reads (program order guarantees correctness)
                for k in range(KT):
                    nc.sync.dma_start(out=wq[k][:], in_=wqkv_s_d[k * P:(k + 1) * P, :])
                for k in range(KT):
                    nc.sync.dma_start(out=wp[k][:], in_=wpr_s_d[k * P:(k + 1) * P, :])

            # =====================================================
            # Stage S
            # =====================================================
            with tc.tile_pool(name="s_sb", bufs=1) as sbS, \
                 tc.tile_pool(name="s_work", bufs=3) as work, \
                 tc.tile_pool(name="s_mm", bufs=3, space="PSUM") as pmm, \
                 tc.tile_pool(name="s_st", bufs=2, space="PSUM") as pst, \
                 tc.tile_pool(name="s_ov", bufs=2, space="PSUM") as pov:

                aS = sbS.tile([P, NT], BF16, tag="aS", name="aS")
                cS = sbS.tile([P, NT], BF16, tag="cS", name="cS")
                lnS = [sbS.tile([P, NS], BF16, tag=f"lnS{k}", name=f"lnS{k}")
                       for k in range(KT)]
                qkS = [sbS.tile([P, NS], BF16, tag=f"qkS{j}", name=f"qkS{j}")
                       for j in range(12)]
                oS = [sbS.tile([P, NS], BF16, tag=f"oS{k}", name=f"oSs{k}")
                      for k in range(KT)]

                lnStok = [sbS.tile([P, NT], BF16, tag=f"lnK{k}", name=f"lnStok{k}")
                          for k in range(KT)]
                for (c0, pc) in CH_T:
                    ln_stats(work, pmm, 1, c0, pc, aS, cS)
                    for k in range(KT):
                        tmp = work.tile([P, 512], BF16, tag="ns", name="ns", bufs=3)
                        nc.vector.tensor_tensor(tmp[:, :pc],
                                                xT[k][:, 1 + c0:1 + c0 + pc],
                                                aS[:, c0:c0 + pc], ALU.mult)
                        nc.vector.tensor_tensor(lnStok[k][:, c0:c0 + pc],
                                                tmp[:, :pc],
                                                cS[:, c0:c0 + pc], ALU.subtract)

                # cls token LN (from the saved original x[1])
                cls_bf = work.tile([P, KT], BF16, tag="clsbf", name="cls_bf")
                nc.vector.tensor_copy(cls_bf[:], cls_save[:])
                psc1 = pmm.tile([P, 512], F32, tag="mm", name="clsx")
                for k in range(KT):
                    nc.tensor.matmul(psc1[0:1, 0:1], ones_sb[:, 0:1],
                                     cls_bf[:, k:k + 1],
                                     start=(k == 0), stop=(k == KT - 1))
                sqc = work.tile([P, KT], BF16, tag="sqc", name="sqc")
                nc.scalar.activation(sqc[:], cls_save[:], AF.Square)
                psc2 = pmm.tile([P, 512], F32, tag="mm", name="clsq")
                for k in range(KT):
                    nc.tensor.matmul(psc2[0:1, 0:1], ones_sb[:, 0:1], sqc[:, k:k + 1],
                                     start=(k == 0), stop=(k == KT - 1))
                muc = work.tile([1, 2], F32, tag="muc", name="muc")
                nc.vector.tensor_scalar_mul(muc[:, 0:1], psc1[0:1, 0:1], INV_D)
                varc = work.tile([1, 1], F32, tag="varc", name="varc")
                nc.vector.scalar_tensor_tensor(varc[:], muc[:, 0:1], -1.0,
                                               muc[:, 0:1], ALU.mult, ALU.mult)
                nc.vector.scalar_tensor_tensor(varc[:], psc2[0:1, 0:1], INV_D,
                                               varc[:], ALU.mult, ALU.add)
                invc = work.tile([1, 1], F32, tag="invc", name="invc")
                nc.scalar.activation(invc[:], varc[:], AF.Ln, bias=eps_sb[0:1])
                nc.scalar.activation(invc[:], invc[:], AF.Exp, scale=-0.5)
                stc = work.tile([P, 2], F32, tag="stc", name="stc")
                nc.gpsimd.partition_broadcast(stc[:, 0:1], muc[:, 0:1])
                nc.gpsimd.partition_broadcast(stc[:, 1:2], invc[:, 0:1])
                lncls = work.tile([P, KT], F32, tag="lncls", name="lncls")
                nc.vector.tensor_scalar(lncls[:], cls_save[:], stc[:, 0:1],
                                        stc[:, 1:2], ALU.subtract, ALU.mult)
                for k in range(KT):
                    nc.vector.tensor_scalar_mul(
                        lnS[k].rearrange("p (t n) -> p t n", t=T)[:, :, 0:1],
                        ones_sb[:, 0:8].rearrange("p (t n) -> p t n", t=T),
                        lncls[:, k:k + 1])

                def fview(ap1568, f):
                    return (ap1568.rearrange("p (w t) -> p t w", t=T)
                            [:, f:f + 1, :].rearrange("p a w -> p (a w)"))

                # scatter token-order lnStok into frame-major lnS, split
                # across ACT and DVE
                for f in range(T):
                    for k in range(KT):
                        src = fview(lnStok[k][:], f)
                        dst = lnS[k][:, f * NF + 1:(f + 1) * NF]
                        if (f * KT + k) % 2 == 0:
                            nc.scalar.activation(dst, src, AF.Copy)
                        else:
                            nc.vector.tensor_copy(dst, src)

                for (f0, nf) in CH_QS:
                    c0, pc = f0 * NF, nf * NF
                    for mi in range(12):
                        ps = pmm.tile([P, 512], F32, tag="mm", name="mmqs")
                        for k in range(KT):
                            nc.tensor.matmul(ps[:, :pc],
                                             wq[k][:, mi * P:(mi + 1) * P],
                                             lnS[k][:, c0:c0 + pc],
                                             start=(k == 0), stop=(k == KT - 1))
                        nc.scalar.activation(qkS[mi][:, c0:c0 + pc], ps[:, :pc],
                                             AF.Identity, bias=Bt[:, BC_QKS + mi:BC_QKS + mi + 1])
                    v_s = {}
                    for f in range(f0, f0 + nf):
                        for i, (k0, pk) in enumerate(tiles_of(NF)):
                            v_s.setdefault(f, {})[i] = sbS.tile(
                                [P, NH, HD + 1], BF16,
                                tag=f"vs{(f % 2) * 2 + i}",
                                name=f"vs{(f % 2) * 2 + i}", bufs=2)
                            nc.vector.memset(v_s[f][i][:pk, :, HD:HD + 1], 1.0)
                            for half in range(2):
                                ps = pmm.tile([P, 512], F32, tag="mm", name="mmvs")
                                for k in range(KT):
                                    nc.tensor.matmul(
                                        ps[:pk, :384],
                                        lnS[k][:, f * NF + k0:f * NF + k0 + pk],
                                        wq[k][:, 2 * D + 384 * half:2 * D + 384 * (half + 1)],
                                        start=(k == 0), stop=(k == KT - 1))
                                nc.vector.scalar_tensor_tensor(
                                    v_s[f][i][:pk, 6 * half:6 * (half + 1), 0:HD],
                                    ps[:pk, :384].rearrange("p (a c) -> p a c", a=6),
                                    1.0,
                                    vc[0:pk, VC_VBS + 384 * half:VC_VBS + 384 * (half + 1)]
                                    .rearrange("p (a c) -> p a c", a=6),
                                    ALU.mult, ALU.add)
                    for f in range(f0, f0 + nf):
                        fc = f * NF
                        for h in range(NH):
                            j, r0 = h // 2, HD * (h % 2)
                            qs = qkS[j][r0:r0 + HD, fc:fc + NF]
                            es_list = []
                            for i, (k0, pk) in enumerate(tiles_of(NF)):
                                st = pst.tile([P, NF], F32, tag="st", name="stS")
                                nc.tensor.matmul(st[:pk, :NF],
                                                 qkS[6 + j][r0:r0 + HD,
                                                            fc + k0:fc + k0 + pk],
                                                 qs, start=True, stop=True)
                                es = work.tile([P, NF], BF16, tag="esS", name="esS",
                                               bufs=4)
                                nc.scalar.activation(es[:pk, :NF], st[:pk, :NF],
                                                     AF.Exp, scale=SCALE)
                                es_list.append((es, k0, pk))
                            ov = pov.tile([HD + 1, NF], F32, tag="ov", name="ovS")
                            for i, (es, k0, pk) in enumerate(es_list):
                                nc.tensor.matmul(ov[:, :NF], v_s[f][i][:pk, h, :],
                                                 es[:pk, :NF], start=(i == 0),
                                                 stop=(i == len(es_list) - 1))
                            rec = work.tile([1, NF], F32, tag="recS", name="recS",
                                            bufs=4)
                            nc.vector.reciprocal(rec[:1, :], ov[HD:HD + 1, :])
                            bc = work.tile([HD, NF], F32, tag="bcS", name="bcS",
                                           bufs=4)
                            nc.gpsimd.partition_broadcast(bc[:, :], rec[0:1, :])
                            nc.vector.tensor_tensor(oS[j][r0:r0 + HD, fc:fc + NF],
                                                    ov[0:HD, :NF], bc[:, :], ALU.mult)

                for (f0, nf) in CH_QS:
                    c0, pc = f0 * NF, nf * NF
                    for mi in range(KT):
                        ps = pmm.tile([P, 512], F32, tag="mm", name="mmps")
                        for k in range(KT):
                            nc.tensor.matmul(ps[:, :pc],
                                             wp[k][:, mi * P:(mi + 1) * P],
                                             oS[k][:, c0:c0 + pc],
                                             start=(k == 0), stop=(k == KT - 1))
                        for f in range(f0, f0 + nf):
                            off = (f - f0) * NF
                            nc.vector.scalar_tensor_tensor(
                                fview(xT[mi][:, 1:1 + NT], f),
                                ps[:, off + 1:off + NF],
                                Bt[:, BC_PRS + mi:BC_PRS + mi + 1],
                                fview(xT[mi][:, 1:1 + NT], f), ALU.add, ALU.add)

                # cls_out = proj(mean over frames of attention-out cls cols)
                oTc = work.tile([P, KT], BF16, tag="oTc", name="oTc")
                for k in range(KT):
                    red = work.tile([P, 1], F32, tag="redc", name="redc", bufs=2)
                    nc.vector.tensor_reduce(
                        red[:],
                        oS[k].rearrange("p (t n) -> p n t", t=T)[:, 0:1, :],
                        AX.X, ALU.add)
                    nc.vector.tensor_scalar_mul(oTc[:, k:k + 1], red[:], 1.0 / T)
                for mi in range(KT):
                    psc = pmm.tile([P, 512], F32, tag="mm", name="clsp")
                    for k in range(KT):
                        nc.tensor.matmul(psc[:, 0:1], wp[k][:, mi * P:(mi + 1) * P],
                                         oTc[:, k:k + 1],
                                         start=(k == 0), stop=(k == KT - 1))
                    nc.vector.scalar_tensor_tensor(
                        xT[mi][:, 0:1], psc[:, 0:1], Bt[:, BC_PRS + mi:BC_PRS + mi + 1],
                        cls_save[:, mi:mi + 1], ALU.add, ALU.add)

                # W2 into the soon-free wq/wp tiles (emitted after last reads)
                w2v = []
                for j in range(24):
                    if j < 18:
                        tgt = wq[j // 3][:, D * (j % 3):D * (j % 3 + 1)]
                    else:
                        tgt = wp[j - 18][:]
                    nc.sync.dma_start(out=tgt, in_=w2_d[j * P:(j + 1) * P, :])
                    w2v.append(tgt)

            # =====================================================
            # Stage M (MLP)
            # =====================================================
            with tc.tile_pool(name="m_sb", bufs=1) as sbM, \
                 tc.tile_pool(name="m_g", bufs=2) as gpool, \
                 tc.tile_pool(name="m_work", bufs=3) as work, \
                 tc.tile_pool(name="m_mm", bufs=3, space="PSUM") as pmm, \
                 tc.tile_pool(name="m_tr", bufs=3, space="PSUM") as ptr:

                aM = sbM.tile([P, N], BF16, tag="aM", name="aM")
                cM = sbM.tile([P, N], BF16, tag="cM", name="cM")
                lnM = [sbM.tile([P, N], BF16, tag=f"lnM{k}", name=f"lnM{k}")
                       for k in range(KT)]
                w1 = [sbM.tile([P, HID], BF16, tag=f"w1{k}", name=f"w1{k}")
                      for k in range(KT)]
                for k in range(KT):
                    nc.sync.dma_start(out=w1[k][:], in_=w1_d[k * P:(k + 1) * P, :])

                # all stats + normalizes first so the ACT table switches
                # ln_exp -> gelu exactly once per iteration
                for (c0, pc) in CH_M:
                    ln_stats(work, pmm, 0, c0, pc, aM, cM)
                    for k in range(KT):
                        tmp = work.tile([P, 512], BF16, tag="nt", name="ntm", bufs=3)
                        nc.vector.tensor_tensor(tmp[:, :pc], xT[k][:, c0:c0 + pc],
                                                aM[:, c0:c0 + pc], ALU.mult)
                        nc.vector.tensor_tensor(lnM[k][:, c0:c0 + pc], tmp[:, :pc],
                                                cM[:, c0:c0 + pc], ALU.subtract)
                for (c0, pc) in CH_M:
                    g1T = [gpool.tile([P, 512], BF16, tag=f"g1T{m}", name=f"g1T{m}")
                           for m in range(24)]
                    for m in range(24):
                        ps = pmm.tile([P, 512], F32, tag="mm", name="f1ps")
                        for k in range(KT):
                            nc.tensor.matmul(ps[:, :pc], w1[k][:, m * P:(m + 1) * P],
                                             lnM[k][:, c0:c0 + pc],
                                             start=(k == 0), stop=(k == KT - 1))
                        if sim_gelu:
                            hb = work.tile([P, 512], F32, tag="hb", name="hb")
                            nc.scalar.activation(hb[:, :pc], ps[:, :pc], AF.Identity,
                                                 bias=Bt[:, BC_FC1 + m:BC_FC1 + m + 1])
                            sg = work.tile([P, 512], F32, tag="sg", name="sg")
                            nc.scalar.activation(sg[:, :pc], hb[:, :pc], AF.Sigmoid,
                                                 scale=1.702)
                            nc.vector.tensor_tensor(g1T[m][:, :pc], hb[:, :pc],
                                                    sg[:, :pc], ALU.mult)
                        else:
                            nc.scalar.activation(g1T[m][:, :pc], ps[:, :pc], AF.Gelu,
                                                 bias=Bt[:, BC_FC1 + m:BC_FC1 + m + 1])
                    for mi in range(KT):
                        ps = pmm.tile([P, 512], F32, tag="mm", name="f2ps")
                        for k in range(24):
                            nc.tensor.matmul(ps[:, :pc], w2v[k][:, mi * P:(mi + 1) * P],
                                             g1T[k][:, :pc],
                                             start=(k == 0), stop=(k == 23))
                        nc.vector.scalar_tensor_tensor(
                            xT[mi][:, c0:c0 + pc], ps[:, :pc], Bt[:, BC_FC2 + mi:BC_FC2 + mi + 1],
                            xT[mi][:, c0:c0 + pc], ALU.add, ALU.add)
                    for (q0, pq) in tiles_of(pc):
                        t0 = c0 + q0
                        out_sb = work.tile([P, D], F32, tag="osb", name="osb", bufs=3)
                        for k in range(KT):
                            ps = ptr.tile([P, P], BF16, tag="tr", name="otr")
                            nc.tensor.transpose(ps[:pq, :], xT[k][:, t0:t0 + pq],
                                                idb[:, :])
                            nc.vector.tensor_copy(out_sb[:pq, k * P:(k + 1) * P],
                                                  ps[:pq, :])
                        nc.sync.dma_start(out=out[t0:t0 + pq, :], in_=out_sb[:pq])

    nc.compile()
    return nc


_CACHED = {}


def _get_program():
    if "nc" not in _CACHED:
        _CACHED["nc"] = build_program()
    return _CACHED["nc"]


def _host_prep(inputs):
    f32 = np.float32
    g = lambda k: np.asarray(inputs[k], f32)
    x = g("x")
    gt, bt = g("gt"), g("bt")
    g1, b1 = g("g1"), g("b1")
    g2, b2 = g("g2"), g("b2")
    Wqkv_t, Wproj_t, bproj_t = g("Wqkv_t"), g("Wproj_t"), g("bproj_t")
    Wqkv_s, Wproj_s, bproj_s = g("Wqkv_s"), g("Wproj_s"), g("bproj_s")
    Wtfc, btfc = g("Wtfc"), g("btfc")
    W1, b1m = g("W1"), g("b1m")
    W2, b2m = g("W2"), g("b2m")

    bf = ml_dtypes.bfloat16
    wqkv_t = np.ascontiguousarray((gt[:, None] * Wqkv_t).astype(bf))
    wqkv_s = np.ascontiguousarray((g1[:, None] * Wqkv_s).astype(bf))
    w1 = np.ascontiguousarray((g2[:, None] * W1).astype(bf))
    qkvb_t = bt @ Wqkv_t
    qkvb_s = b1 @ Wqkv_s
    b1m_f = b2 @ W1 + b1m

    def cols(vec, n):
        return np.asarray(vec, f32).reshape(n, P).T

    biases = np.concatenate([
        cols(qkvb_t[:2 * D], 12), cols(bproj_t, 6), cols(btfc, 6),
        cols(qkvb_s[:2 * D], 12), cols(bproj_s, 6),
        cols(b1m_f, 24), cols(b2m, 6)], axis=1).astype(f32)

    mask = np.kron(np.eye(16, dtype=f32), np.ones((8, 8), f32))
    ident = np.eye(P, dtype=f32)
    vb_t = np.tile(qkvb_t[2 * D:], (P, 1))
    vb_s = np.tile(qkvb_s[2 * D:], (P, 1))
    vconst = np.concatenate([mask, ident, vb_t, vb_s], axis=1).astype(bf)

    base = {
        "wqkv_t": wqkv_t, "wpr_t": np.ascontiguousarray(Wproj_t.astype(bf)),
        "wtfc": np.ascontiguousarray(Wtfc.astype(bf)),
        "wqkv_s": wqkv_s, "wpr_s": np.ascontiguousarray(Wproj_s.astype(bf)),
        "w1": w1, "w2": np.ascontiguousarray(W2.astype(bf)),
        "biases": np.ascontiguousarray(biases),
        "vconst": np.ascontiguousarray(vconst),
    }
    maps = []
    for i in range(B):
        xb = np.zeros((NPAD, D), bf)
        xb[:N] = x[i].astype(bf)
        maps.append(dict(base, xbf=np.ascontiguousarray(xb)))
    return maps


def make_in_maps(inputs):
    return _host_prep(inputs)


def kernel(**inputs):
    nc = _get_program()
    in_maps = make_in_maps(inputs)
    core_ids = list(range(8))
    from concourse.bass_utils import run_bass_kernel_spmd
    res = run_bass_kernel_spmd(nc, in_maps, core_ids)
    return np.stack([res.results[i]["out"] for i in core_ids], axis=0)


if __name__ == "__main__":
    build_program()
    print("built ok")


# revision 6
# speedup vs baseline: 1.2818x; 1.0549x over previous
"""Trainium2 Bass kernel v2 for the TimeSformer-style divided space-time block.

Data-parallel over B (8 cores). Per core, the residual stream lives in SBUF
feature-major as bf16 for the whole block:

  - x loaded once via DMA-transpose (host pre-casts x to bf16, pads to 1664
    rows); no DRAM round trips between the three stages.
  - LayerNorm is computed feature-major: Sum(x) / Sum(x^2) via ones-matmuls on
    the PE (bf16), ACT Square for x^2, then a 2-pass DVE normalize with
    per-token scale/offset rows partition-broadcast by GpSimd.
  - LN's gamma is folded into the following weight matrix on the host;
    LN's beta contributes b@W which is applied as a per-feature bias at
    eviction (q,k) or through the V columns (softmax rows sum to 1).
  - Branch outputs are accumulated into the residual directly from PSUM with
    fused scalar_tensor_tensor evictions (one rounding per residual add).
  - Spatial attention runs on a frame-major copy of the normalized stream
    (strided per-frame normalize); temporal attention runs in token order
    with the S^T block-diagonal mask trick.
"""

import numpy as np
import ml_dtypes

import concourse.bass as bass
import concourse.mybir as mybir
import concourse.tile as tile
from concourse import bacc

F32 = mybir.dt.float32
BF16 = mybir.dt.bfloat16
AF = mybir.ActivationFunctionType
ALU = mybir.AluOpType
AX = mybir.AxisListType

D = 768
KT = 6
NH = 12
HD = 64
HID = 3072
B = 8
T = 8
HWn = 196
N = 1569
NPAD = 1664
NT = 1568
NF = 197
NS = T * NF
SCALE = HD ** -0.5
P = 128
EPS = 1e-5
INV_D = 1.0 / D

CH_T = [(0, 128), (128, 512), (640, 512), (1152, 416)]
CH_M = [(0, 128), (128, 512), (640, 512), (1152, 417)]
CH_QS = [(0, 1), (1, 1), (2, 2), (4, 2), (6, 2)]

BC_QKT = 0
BC_PRT = 12
BC_TFC = 18
BC_QKS = 24
BC_PRS = 36
BC_FC1 = 42
BC_FC2 = 66

VC_MASK = 0
VC_ID = 128
VC_VBT = 256
VC_VBS = 1024


def tiles_of(n, step=128):
    return [(i, min(step, n - i)) for i in range(0, n, step)]


def build_program(loop_n=0, sim_gelu=False):
    nc = bacc.Bacc("TRN2", target_bir_lowering=False, debug=False, num_devices=8)

    xbf = nc.dram_tensor("xbf", [NPAD, D], BF16, kind="ExternalInput").ap()
    wqkv_t_d = nc.dram_tensor("wqkv_t", [D, 3 * D], BF16, kind="ExternalInput").ap()
    wpr_t_d = nc.dram_tensor("wpr_t", [D, D], BF16, kind="ExternalInput").ap()
    wtfc_d = nc.dram_tensor("wtfc", [D, D], BF16, kind="ExternalInput").ap()
    wqkv_s_d = nc.dram_tensor("wqkv_s", [D, 3 * D], BF16, kind="ExternalInput").ap()
    wpr_s_d = nc.dram_tensor("wpr_s", [D, D], BF16, kind="ExternalInput").ap()
    w1_d = nc.dram_tensor("w1", [D, HID], BF16, kind="ExternalInput").ap()
    w2_d = nc.dram_tensor("w2", [HID, D], BF16, kind="ExternalInput").ap()
    biases_d = nc.dram_tensor("biases", [P, 72], F32, kind="ExternalInput").ap()
    vconst_d = nc.dram_tensor("vconst", [P, 1792], BF16, kind="ExternalInput").ap()
    out = nc.dram_tensor("out", [N, D], F32, kind="ExternalOutput").ap()

    from contextlib import nullcontext

    with tile.TileContext(nc) as tc:
      with tc.tile_pool(name="const", bufs=1) as const:
        eps_sb = const.tile([P, 1], F32, tag="eps")
        nc.vector.memset(eps_sb[:], EPS)
        ones_sb = const.tile([P, 8], BF16, tag="ones")
        nc.vector.memset(ones_sb[:], 1.0)
        oneD_sb = const.tile([P, 1], BF16, tag="oneD")
        nc.vector.memset(oneD_sb[:], INV_D)

        loop_cm = tc.For_i(0, loop_n, 1) if loop_n else nullcontext()
        with loop_cm:
          with tc.tile_pool(name="glob", bufs=1) as glob:
            xT = [glob.tile([P, NPAD], BF16, tag=f"xT{k}", name=f"xT{k}")
                  for k in range(KT)]
            for k in range(KT):
                nc.sync.dma_start(out=xT[k][:], in_=xbf[:, k * P:(k + 1) * P],
                                  transpose=True)
            Bt = glob.tile([P, 72], F32, tag="biases", name="biases")
            nc.sync.dma_start(out=Bt[:], in_=biases_d)
            vc = glob.tile([P, 1792], BF16, tag="vconst", name="vconst")
            nc.sync.dma_start(out=vc[:], in_=vconst_d)
            mask = vc[:, VC_MASK:VC_MASK + P]
            idb = vc[:, VC_ID:VC_ID + P]

            wq = [glob.tile([P, 3 * D], BF16, tag=f"wq{k}", name=f"wq{k}")
                  for k in range(KT)]
            wp = [glob.tile([P, D], BF16, tag=f"wp{k}", name=f"wp{k}")
                  for k in range(KT)]
            for k in range(KT):
                nc.sync.dma_start(out=wq[k][:], in_=wqkv_t_d[k * P:(k + 1) * P, :])
            for k in range(KT):
                nc.sync.dma_start(out=wp[k][:], in_=wpr_t_d[k * P:(k + 1) * P, :])

            cls_save = glob.tile([P, KT], F32, tag="cls", name="cls_save")
            for k in range(KT):
                nc.vector.tensor_copy(cls_save[:, k:k + 1], xT[k][:, 1:2])

            def ln_stats(pool, ps_pool, src_col0, c0, pc, a_bc, c_bc):
                """Per-token scale/offset rows for xT cols [src_col0+c0, +pc)."""
                # ones vector pre-scaled by 1/D: psA = mean, psB = E[x^2]
                psA = ps_pool.tile([P, 512], F32, tag="mm", name="sx")
                for k in range(KT):
                    src = xT[k][:, src_col0 + c0:src_col0 + c0 + pc]
                    nc.tensor.matmul(psA[0:1, :pc], oneD_sb[:, 0:1], src,
                                     start=(k == 0), stop=(k == KT - 1))
                psB = ps_pool.tile([P, 512], F32, tag="mm", name="sq")
                for k in range(KT):
                    src = xT[k][:, src_col0 + c0:src_col0 + c0 + pc]
                    sq = pool.tile([P, 512], BF16, tag="sqv", name="sqv", bufs=2)
                    nc.scalar.activation(sq[:, :pc], src, AF.Square)
                    nc.tensor.matmul(psB[0:1, :pc], oneD_sb[:, 0:1], sq[:, :pc],
                                     start=(k == 0), stop=(k == KT - 1))
                mu = pool.tile([1, 512], F32, tag="mu", name="mu", bufs=2)
                nc.vector.tensor_copy(mu[:, :pc], psA[0:1, :pc])
                r2 = pool.tile([1, 512], F32, tag="r2", name="r2", bufs=2)
                # var = E[x^2] - mu^2
                nc.vector.scalar_tensor_tensor(r2[:, :pc], mu[:, :pc], -1.0,
                                               mu[:, :pc], ALU.mult, ALU.mult)
                nc.vector.tensor_tensor(r2[:, :pc], psB[0:1, :pc], r2[:, :pc],
                                        ALU.add)
                # inv = exp(-0.5*ln(var+eps)); Ln and Exp live in the same ACT
                # table as Square/Identity (natural_log_exp_and_others)
                nc.scalar.activation(r2[:, :pc], r2[:, :pc], AF.Ln,
                                     bias=eps_sb[0:1])
                a_row = pool.tile([1, 512], BF16, tag="arow", name="arow", bufs=2)
                nc.scalar.activation(a_row[:, :pc], r2[:, :pc], AF.Exp, scale=-0.5)
                c_row = pool.tile([1, 512], BF16, tag="crow", name="crow", bufs=2)
                nc.vector.tensor_tensor(c_row[:, :pc], mu[:, :pc],
                                        a_row[:, :pc], ALU.mult)
                nc.gpsimd.partition_broadcast(a_bc[:, c0:c0 + pc], a_row[0:1, :pc])
                nc.gpsimd.partition_broadcast(c_bc[:, c0:c0 + pc], c_row[0:1, :pc])

            # =====================================================
            # Stage T
            # =====================================================
            with tc.tile_pool(name="t_sb", bufs=1) as sbT, \
                 tc.tile_pool(name="t_work", bufs=3) as work, \
                 tc.tile_pool(name="t_mm", bufs=3, space="PSUM") as pmm, \
                 tc.tile_pool(name="t_att", bufs=3, space="PSUM") as patt, \
                 tc.tile_pool(name="t_tr", bufs=2, space="PSUM") as ptr:

                wtfc = [sbT.tile([P, D], BF16, tag=f"wt{k}", name=f"wt{k}")
                        for k in range(KT)]
                for k in range(KT):
                    nc.sync.dma_start(out=wtfc[k][:], in_=wtfc_d[k * P:(k + 1) * P, :])

                aT = sbT.tile([P, NT], BF16, tag="aT", name="aT")
                cT = sbT.tile([P, NT], BF16, tag="cT", name="cT")
                lnT = [sbT.tile([P, NT], BF16, tag=f"ln{k}", name=f"lnT{k}")
                       for k in range(KT)]
                qkT = [sbT.tile([P, NT], BF16, tag=f"qk{j}", name=f"qkT{j}")
                       for j in range(12)]
                v_t = [sbT.tile([P, NH, HD + 1], BF16, tag=f"vt{g}", name=f"vt{g}")
                       for g in range(13)]
                oT = [sbT.tile([P, NT], BF16, tag=f"oT{k}", name=f"oTt{k}")
                      for k in range(KT)]

                for (c0, pc) in CH_T:
                    ln_stats(work, pmm, 1, c0, pc, aT, cT)
                    for k in range(KT):
                        tmp = work.tile([P, 512], BF16, tag="nt", name="nt", bufs=3)
                        nc.vector.tensor_tensor(tmp[:, :pc],
                                                xT[k][:, 1 + c0:1 + c0 + pc],
                                                aT[:, c0:c0 + pc], ALU.mult)
                        nc.vector.tensor_tensor(lnT[k][:, c0:c0 + pc], tmp[:, :pc],
                                                cT[:, c0:c0 + pc], ALU.subtract)
                    for mi in range(12):
                        ps = pmm.tile([P, 512], F32, tag="mm", name="mm")
                        for k in range(KT):
                            nc.tensor.matmul(ps[:, :pc],
                                             wq[k][:, mi * P:(mi + 1) * P],
                                             lnT[k][:, c0:c0 + pc],
                                             start=(k == 0), stop=(k == KT - 1))
                        nc.scalar.activation(qkT[mi][:, c0:c0 + pc], ps[:, :pc],
                                             AF.Identity, bias=Bt[:, BC_QKT + mi:BC_QKT + mi + 1])
                    for (g0, gp) in tiles_of(pc):
                        g = (c0 + g0) // P
                        t0 = c0 + g0
                        nc.vector.memset(v_t[g][:gp, :, HD:HD + 1], 1.0)
                        for half in range(2):
                            ps = pmm.tile([P, 512], F32, tag="mm", name="mmv")
                            for k in range(KT):
                                nc.tensor.matmul(
                                    ps[:gp, :384],
                                    lnT[k][:, t0:t0 + gp],
                                    wq[k][:, 2 * D + 384 * half:2 * D + 384 * (half + 1)],
                                    start=(k == 0), stop=(k == KT - 1))
                            nc.vector.scalar_tensor_tensor(
                                v_t[g][:gp, 6 * half:6 * (half + 1), 0:HD],
                                ps[:gp, :384].rearrange("p (a c) -> p a c", a=6),
                                1.0,
                                vc[0:gp, VC_VBT + 384 * half:VC_VBT + 384 * (half + 1)]
                                .rearrange("p (a c) -> p a c", a=6),
                                ALU.mult, ALU.add)
                    for (g0, gp) in tiles_of(pc):
                        g = (c0 + g0) // P
                        t0 = c0 + g0
                        o_tm = work.tile([P, D], BF16, tag="otm", name="otm", bufs=2)
                        for h in range(NH):
                            j, r0 = h // 2, HD * (h % 2)
                            att = patt.tile([P, P + HD + 1], F32, tag="att",
                                            name="att")
                            st = att[:, 0:P]
                            ov = att[:, P:P + HD + 1]
                            nc.tensor.matmul(st[:gp, :gp],
                                             qkT[6 + j][r0:r0 + HD, t0:t0 + gp],
                                             qkT[j][r0:r0 + HD, t0:t0 + gp],
                                             start=True, stop=True)
                            es = work.tile([P, P], BF16, tag="es", name="es", bufs=4)
                            nc.scalar.activation(es[:gp, :gp], st[:gp, :gp], AF.Exp,
                                                 scale=SCALE)
                            nc.gpsimd.tensor_tensor(es[:gp, :gp], es[:gp, :gp],
                                                    mask[0:gp, 0:gp], ALU.mult)
                            nc.tensor.matmul(ov[:gp, :], es[:gp, :gp],
                                             v_t[g][:gp, h, :], start=True, stop=True)
                            rec = work.tile([P, 1], F32, tag="rec", name="rec", bufs=4)
                            nc.vector.reciprocal(rec[:gp], ov[:gp, HD:HD + 1])
                            nc.vector.tensor_scalar_mul(o_tm[:gp, HD * h:HD * (h + 1)],
                                                        ov[:gp, 0:HD], rec[:gp])
                        for k in range(KT):
                            ps = ptr.tile([P, P], BF16, tag="tr", name="tr")
                            nc.tensor.transpose(ps[:, :gp], o_tm[:gp, k * P:(k + 1) * P],
                                                idb[0:gp, 0:gp])
                            nc.vector.tensor_copy(oT[k][:, t0:t0 + gp], ps[:, :gp])

                # proj -> pT (chunk-local) -> tfc -> residual accumulate
                for (c0, pc) in CH_T:
                    pTc = []
                    for mi in range(KT):
                        ps = pmm.tile([P, 512], F32, tag="mm", name="mmp")
                        for k in range(KT):
                            nc.tensor.matmul(ps[:, :pc],
                                             wp[k][:, mi * P:(mi + 1) * P],
                                             oT[k][:, c0:c0 + pc],
                                             start=(k == 0), stop=(k == KT - 1))
                        pT = work.tile([P, 512], BF16, tag=f"pT{mi}",
                                       name=f"pT{mi}", bufs=2)
                        nc.scalar.activation(pT[:, :pc], ps[:, :pc],
                                             AF.Identity, bias=Bt[:, BC_PRT + mi:BC_PRT + mi + 1])
                        pTc.append(pT)
                    for mi in range(KT):
                        ps = pmm.tile([P, 512], F32, tag="mm", name="mmt")
                        for k in range(KT):
                            nc.tensor.matmul(ps[:, :pc],
                                             wtfc[k][:, mi * P:(mi + 1) * P],
                                             pTc[k][:, :pc],
                                             start=(k == 0), stop=(k == KT - 1))
                        nc.vector.scalar_tensor_tensor(
                            xT[mi][:, 1 + c0:1 + c0 + pc], ps[:, :pc],
                            Bt[:, BC_TFC + mi:BC_TFC + mi + 1],
                            xT[mi][:, 1 + c0:1 + c0 + pc], ALU.add, ALU.add)

                # spatial weights go into the same tiles, after the last
                # temporal reads (program order guarantees correctness)
                for k in range(KT):
                    nc.sync.dma_start(out=wq[k][:], in_=wqkv_s_d[k * P:(k + 1) * P, :])
                for k in range(KT):
                    nc.sync.dma_start(out=wp[k][:], in_=wpr_s_d[k * P:(k + 1) * P, :])

            # =====================================================
            # Stage S
            # =====================================================
            with tc.tile_pool(name="s_sb", bufs=1) as sbS, \
                 tc.tile_pool(name="s_work", bufs=3) as work, \
                 tc.tile_pool(name="s_mm", bufs=3, space="PSUM") as pmm, \
                 tc.tile_pool(name="s_st", bufs=2, space="PSUM") as pst, \
                 tc.tile_pool(name="s_ov", bufs=2, space="PSUM") as pov:

                aS = sbS.tile([P, NT], BF16, tag="aS", name="aS")
                cS = sbS.tile([P, NT], BF16, tag="cS", name="cS")
                lnS = [sbS.tile([P, NS], BF16, tag=f"lnS{k}", name=f"lnS{k}")
                       for k in range(KT)]
                qkS = [sbS.tile([P, NS], BF16, tag=f"qkS{j}", name=f"qkS{j}")
                       for j in range(12)]
                oS = [sbS.tile([P, NS], BF16, tag=f"oS{k}", name=f"oSs{k}")
                      for k in range(KT)]

                lnStok = [sbS.tile([P, NT], BF16, tag=f"lnK{k}", name=f"lnStok{k}")
                          for k in range(KT)]
                for (c0, pc) in CH_T:
                    ln_stats(work, pmm, 1, c0, pc, aS, cS)
                    for k in range(KT):
                        tmp = work.tile([P, 512], BF16, tag="ns", name="ns", bufs=3)
                        nc.vector.tensor_tensor(tmp[:, :pc],
                                                xT[k][:, 1 + c0:1 + c0 + pc],
                                                aS[:, c0:c0 + pc], ALU.mult)
                        nc.vector.tensor_tensor(lnStok[k][:, c0:c0 + pc],
                                                tmp[:, :pc],
                                                cS[:, c0:c0 + pc], ALU.subtract)

                # cls token LN (from the saved original x[1])
                cls_bf = work.tile([P, KT], BF16, tag="clsbf", name="cls_bf")
                nc.vector.tensor_copy(cls_bf[:], cls_save[:])
                psc1 = pmm.tile([P, 512], F32, tag="mm", name="clsx")
                for k in range(KT):
                    nc.tensor.matmul(psc1[0:1, 0:1], ones_sb[:, 0:1],
                                     cls_bf[:, k:k + 1],
                                     start=(k == 0), stop=(k == KT - 1))
                sqc = work.tile([P, KT], BF16, tag="sqc", name="sqc")
                nc.scalar.activation(sqc[:], cls_save[:], AF.Square)
                psc2 = pmm.tile([P, 512], F32, tag="mm", name="clsq")
                for k in range(KT):
                    nc.tensor.matmul(psc2[0:1, 0:1], ones_sb[:, 0:1], sqc[:, k:k + 1],
                                     start=(k == 0), stop=(k == KT - 1))
                muc = work.tile([1, 2], F32, tag="muc", name="muc")
                nc.vector.tensor_scalar_mul(muc[:, 0:1], psc1[0:1, 0:1], INV_D)
                varc = work.tile([1, 1], F32, tag="varc", name="varc")
                nc.vector.scalar_tensor_tensor(varc[:], muc[:, 0:1], -1.0,
                                               muc[:, 0:1], ALU.mult, ALU.mult)
                nc.vector.scalar_tensor_tensor(varc[:], psc2[0:1, 0:1], INV_D,
                                               varc[:], ALU.mult, ALU.add)
                invc = work.tile([1, 1], F32, tag="invc", name="invc")
                nc.scalar.activation(invc[:], varc[:], AF.Ln, bias=eps_sb[0:1])
                nc.scalar.activation(invc[:], invc[:], AF.Exp, scale=-0.5)
                stc = work.tile([P, 2], F32, tag="stc", name="stc")
                nc.gpsimd.partition_broadcast(stc[:, 0:1], muc[:, 0:1])
                nc.gpsimd.partition_broadcast(stc[:, 1:2], invc[:, 0:1])
                lncls = work.tile([P, KT], F32, tag="lncls", name="lncls")
                nc.vector.tensor_scalar(lncls[:], cls_save[:], stc[:, 0:1],
                                        stc[:, 1:2], ALU.subtract, ALU.mult)
                for k in range(KT):
                    nc.vector.tensor_scalar_mul(
                        lnS[k].rearrange("p (t n) -> p t n", t=T)[:, :, 0:1],
                        ones_sb[:, 0:8].rearrange("p (t n) -> p t n", t=T),
                        lncls[:, k:k + 1])

                def fview(ap1568, f):
                    return (ap1568.rearrange("p (w t) -> p t w", t=T)
                            [:, f:f + 1, :].rearrange("p a w -> p (a w)"))

                # scatter token-order lnStok into frame-major lnS, split
                # across ACT and DVE
                for f in range(T):
                    for k in range(KT):
                        src = fview(lnStok[k][:], f)
                        dst = lnS[k][:, f * NF + 1:(f + 1) * NF]
                        if (f * KT + k) % 2 == 0:
                            nc.scalar.activation(dst, src, AF.Copy)
                        else:
                            nc.vector.tensor_copy(dst, src)

                for (f0, nf) in CH_QS:
                    c0, pc = f0 * NF, nf * NF
                    for mi in range(12):
                        ps = pmm.tile([P, 512], F32, tag="mm", name="mmqs")
                        for k in range(KT):
                            nc.tensor.matmul(ps[:, :pc],
                                             wq[k][:, mi * P:(mi + 1) * P],
                                             lnS[k][:, c0:c0 + pc],
                                             start=(k == 0), stop=(k == KT - 1))
                        nc.scalar.activation(qkS[mi][:, c0:c0 + pc], ps[:, :pc],
                                             AF.Identity, bias=Bt[:, BC_QKS + mi:BC_QKS + mi + 1])
                    v_s = {}
                    for f in range(f0, f0 + nf):
                        for i, (k0, pk) in enumerate(tiles_of(NF)):
                            v_s.setdefault(f, {})[i] = sbS.tile(
                                [P, NH, HD + 1], BF16,
                                tag=f"vs{(f % 2) * 2 + i}",
                                name=f"vs{(f % 2) * 2 + i}", bufs=2)
                            nc.vector.memset(v_s[f][i][:pk, :, HD:HD + 1], 1.0)
                            for half in range(2):
                                ps = pmm.tile([P, 512], F32, tag="mm", name="mmvs")
                                for k in range(KT):
                                    nc.tensor.matmul(
                                        ps[:pk, :384],
                                        lnS[k][:, f * NF + k0:f * NF + k0 + pk],
                                        wq[k][:, 2 * D + 384 * half:2 * D + 384 * (half + 1)],
                                        start=(k == 0), stop=(k == KT - 1))
                                nc.vector.scalar_tensor_tensor(
                                    v_s[f][i][:pk, 6 * half:6 * (half + 1), 0:HD],
                                    ps[:pk, :384].rearrange("p (a c) -> p a c", a=6),
                                    1.0,
                                    vc[0:pk, VC_VBS + 384 * half:VC_VBS + 384 * (half + 1)]
                                    .rearrange("p (a c) -> p a c", a=6),
                                    ALU.mult, ALU.add)
                    for f in range(f0, f0 + nf):
                        fc = f * NF
                        for h in range(NH):
                            j, r0 = h // 2, HD * (h % 2)
                            qs = qkS[j][r0:r0 + HD, fc:fc + NF]
                            es_list = []
                            for i, (k0, pk) in enumerate(tiles_of(NF)):
                                st = pst.tile([P, NF], F32, tag="st", name="stS")
                                nc.tensor.matmul(st[:pk, :NF],
                                                 qkS[6 + j][r0:r0 + HD,
                                                            fc + k0:fc + k0 + pk],
                                                 qs, start=True, stop=True)
                                es = work.tile([P, NF], BF16, tag="esS", name="esS",
                                               bufs=4)
                                nc.scalar.activation(es[:pk, :NF], st[:pk, :NF],
                                                     AF.Exp, scale=SCALE)
                                es_list.append((es, k0, pk))
                            ov = pov.tile([HD + 1, NF], F32, tag="ov", name="ovS")
                            for i, (es, k0, pk) in enumerate(es_list):
                                nc.tensor.matmul(ov[:, :NF], v_s[f][i][:pk, h, :],
                                                 es[:pk, :NF], start=(i == 0),
                                                 stop=(i == len(es_list) - 1))
                            rec = work.tile([1, NF], F32, tag="recS", name="recS",
                                            bufs=4)
                            nc.vector.reciprocal(rec[:1, :], ov[HD:HD + 1, :])
                            bc = work.tile([HD, NF], F32, tag="bcS", name="bcS",
                                           bufs=4)
                            nc.gpsimd.partition_broadcast(bc[:, :], rec[0:1, :])
                            nc.vector.tensor_tensor(oS[j][r0:r0 + HD, fc:fc + NF],
                                                    ov[0:HD, :NF], bc[:, :], ALU.mult)

                for (f0, nf) in CH_QS:
                    c0, pc = f0 * NF, nf * NF
                    for mi in range(KT):
                        ps = pmm.tile([P, 512], F32, tag="mm", name="mmps")
                        for k in range(KT):
                            nc.tensor.matmul(ps[:, :pc],
                                             wp[k][:, mi * P:(mi + 1) * P],
                                             oS[k][:, c0:c0 + pc],
                                             start=(k == 0), stop=(k == KT - 1))
                        for f in range(f0, f0 + nf):
                            off = (f - f0) * NF
                            nc.vector.scalar_tensor_tensor(
                                fview(xT[mi][:, 1:1 + NT], f),
                                ps[:, off + 1:off + NF],
                                Bt[:, BC_PRS + mi:BC_PRS + mi + 1],
                                fview(xT[mi][:, 1:1 + NT], f), ALU.add, ALU.add)

                # cls_out = proj(mean over frames of attention-out cls cols)
                oTc = work.tile([P, KT], BF16, tag="oTc", name="oTc")
                for k in range(KT):
                    red = work.tile([P, 1], F32, tag="redc", name="redc", bufs=2)
                    nc.vector.tensor_reduce(
                        red[:],
                        oS[k].rearrange("p (t n) -> p n t", t=T)[:, 0:1, :],
                        AX.X, ALU.add)
                    nc.vector.tensor_scalar_mul(oTc[:, k:k + 1], red[:], 1.0 / T)
                for mi in range(KT):
                    psc = pmm.tile([P, 512], F32, tag="mm", name="clsp")
                    for k in range(KT):
                        nc.tensor.matmul(psc[:, 0:1], wp[k][:, mi * P:(mi + 1) * P],
                                         oTc[:, k:k + 1],
                                         start=(k == 0), stop=(k == KT - 1))
                    nc.vector.scalar_tensor_tensor(
                        xT[mi][:, 0:1], psc[:, 0:1], Bt[:, BC_PRS + mi:BC_PRS + mi + 1],
                        cls_save[:, mi:mi + 1], ALU.add, ALU.add)

                # W2 into the soon-free wq/wp tiles (emitted after last reads)
                w2v = []
                for j in range(24):
                    if j < 18:
                        tgt = wq[j // 3][:, D * (j % 3):D * (j % 3 + 1)]
                    else:
                        tgt = wp[j - 18][:]
                    nc.sync.dma_start(out=tgt, in_=w2_d[j * P:(j + 1) * P, :])
                    w2v.append(tgt)

            # =====================================================
            # Stage M (MLP)
            # =====================================================
            with tc.tile_pool(name="m_sb", bufs=1) as sbM, \
                 tc.tile_pool(name="m_g", bufs=2) as gpool, \
                 tc.tile_pool(name="m_work", bufs=3) as work, \
                 tc.tile_pool(name="m_mm", bufs=3, space="PSUM") as pmm, \
                 tc.tile_pool(name="m_tr", bufs=3, space="PSUM") as ptr:

                aM = sbM.tile([P, N], BF16, tag="aM", name="aM")
                cM = sbM.tile([P, N], BF16, tag="cM", name="cM")
                lnM = [sbM.tile([P, N], BF16, tag=f"lnM{k}", name=f"lnM{k}")
                       for k in range(KT)]
                w1 = [sbM.tile([P, HID], BF16, tag=f"w1{k}", name=f"w1{k}")
                      for k in range(KT)]
                for k in range(KT):
                    nc.sync.dma_start(out=w1[k][:], in_=w1_d[k * P:(k + 1) * P, :])

                # all stats + normalizes first so the ACT table switches
                # ln_exp -> gelu exactly once per iteration
                for (c0, pc) in CH_M:
                    ln_stats(work, pmm, 0, c0, pc, aM, cM)
                    for k in range(KT):
                        tmp = work.tile([P, 512], BF16, tag="nt", name="ntm", bufs=3)
                        nc.vector.tensor_tensor(tmp[:, :pc], xT[k][:, c0:c0 + pc],
                                                aM[:, c0:c0 + pc], ALU.mult)
                        nc.vector.tensor_tensor(lnM[k][:, c0:c0 + pc], tmp[:, :pc],
                                                cM[:, c0:c0 + pc], ALU.subtract)
                for (c0, pc) in CH_M:
                    g1T = [gpool.tile([P, 512], BF16, tag=f"g1T{m}", name=f"g1T{m}")
                           for m in range(24)]
                    for m in range(24):
                        ps = pmm.tile([P, 512], F32, tag="mm", name="f1ps")
                        for k in range(KT):
                            nc.tensor.matmul(ps[:, :pc], w1[k][:, m * P:(m + 1) * P],
                                             lnM[k][:, c0:c0 + pc],
                                             start=(k == 0), stop=(k == KT - 1))
                        if sim_gelu:
                            hb = work.tile([P, 512], F32, tag="hb", name="hb", bufs=1)
                            nc.scalar.activation(hb[:, :pc], ps[:, :pc], AF.Identity,
                                                 bias=Bt[:, BC_FC1 + m:BC_FC1 + m + 1])
                            sg = work.tile([P, 512], F32, tag="sg", name="sg", bufs=1)
                            nc.scalar.activation(sg[:, :pc], hb[:, :pc], AF.Sigmoid,
                                                 scale=1.702)
                            nc.vector.tensor_tensor(g1T[m][:, :pc], hb[:, :pc],
                                                    sg[:, :pc], ALU.mult)
                        else:
                            nc.scalar.activation(g1T[m][:, :pc], ps[:, :pc], AF.Gelu,
                                                 bias=Bt[:, BC_FC1 + m:BC_FC1 + m + 1])
                    for mi in range(KT):
                        ps = pmm.tile([P, 512], F32, tag="mm", name="f2ps")
                        for k in range(24):
                            nc.tensor.matmul(ps[:, :pc], w2v[k][:, mi * P:(mi + 1) * P],
                                             g1T[k][:, :pc],
                                             start=(k == 0), stop=(k == 23))
                        nc.vector.scalar_tensor_tensor(
                            xT[mi][:, c0:c0 + pc], ps[:, :pc], Bt[:, BC_FC2 + mi:BC_FC2 + mi + 1],
                            xT[mi][:, c0:c0 + pc], ALU.add, ALU.add)
                    for (q0, pq) in tiles_of(pc):
                        t0 = c0 + q0
                        out_sb = work.tile([P, D], F32, tag="osb", name="osb", bufs=3)
                        for k in range(KT):
                            ps = ptr.tile([P, P], BF16, tag="tr", name="otr")
                            nc.tensor.transpose(ps[:pq, :], xT[k][:, t0:t0 + pq],
                                                idb[:, :])
                            nc.vector.tensor_copy(out_sb[:pq, k * P:(k + 1) * P],
                                                  ps[:pq, :])
                        nc.sync.dma_start(out=out[t0:t0 + pq, :], in_=out_sb[:pq])

    nc.compile()
    return nc


_CACHED = {}


def _get_program():
    if "nc" not in _CACHED:
        _CACHED["nc"] = build_program()
    return _CACHED["nc"]


def _host_prep(inputs):
    f32 = np.float32
    g = lambda k: np.asarray(inputs[k], f32)
    x = g("x")
    gt, bt = g("gt"), g("bt")
    g1, b1 = g("g1"), g("b1")
    g2, b2 = g("g2"), g("b2")
    Wqkv_t, Wproj_t, bproj_t = g("Wqkv_t"), g("Wproj_t"), g("bproj_t")
    Wqkv_s, Wproj_s, bproj_s = g("Wqkv_s"), g("Wproj_s"), g("bproj_s")
    Wtfc, btfc = g("Wtfc"), g("btfc")
    W1, b1m = g("W1"), g("b1m")
    W2, b2m = g("W2"), g("b2m")

    bf = ml_dtypes.bfloat16
    wqkv_t = np.ascontiguousarray((gt[:, None] * Wqkv_t).astype(bf))
    wqkv_s = np.ascontiguousarray((g1[:, None] * Wqkv_s).astype(bf))
    w1 = np.ascontiguousarray((g2[:, None] * W1).astype(bf))
    qkvb_t = bt @ Wqkv_t
    qkvb_s = b1 @ Wqkv_s
    b1m_f = b2 @ W1 + b1m

    def cols(vec, n):
        return np.asarray(vec, f32).reshape(n, P).T

    biases = np.concatenate([
        cols(qkvb_t[:2 * D], 12), cols(bproj_t, 6), cols(btfc, 6),
        cols(qkvb_s[:2 * D], 12), cols(bproj_s, 6),
        cols(b1m_f, 24), cols(b2m, 6)], axis=1).astype(f32)

    mask = np.kron(np.eye(16, dtype=f32), np.ones((8, 8), f32))
    ident = np.eye(P, dtype=f32)
    vb_t = np.tile(qkvb_t[2 * D:], (P, 1))
    vb_s = np.tile(qkvb_s[2 * D:], (P, 1))
    vconst = np.concatenate([mask, ident, vb_t, vb_s], axis=1).astype(bf)

    base = {
        "wqkv_t": wqkv_t, "wpr_t": np.ascontiguousarray(Wproj_t.astype(bf)),
        "wtfc": np.ascontiguousarray(Wtfc.astype(bf)),
        "wqkv_s": wqkv_s, "wpr_s": np.ascontiguousarray(Wproj_s.astype(bf)),
        "w1": w1, "w2": np.ascontiguousarray(W2.astype(bf)),
        "biases": np.ascontiguousarray(biases),
        "vconst": np.ascontiguousarray(vconst),
    }
    maps = []
    for i in range(B):
        xb = np.zeros((NPAD, D), bf)
        xb[:N] = x[i].astype(bf)
        maps.append(dict(base, xbf=np.ascontiguousarray(xb)))
    return maps


def make_in_maps(inputs):
    return _host_prep(inputs)


def kernel(**inputs):
    nc = _get_program()
    in_maps = make_in_maps(inputs)
    core_ids = list(range(8))
    from concourse.bass_utils import run_bass_kernel_spmd
    res = run_bass_kernel_spmd(nc, in_maps, core_ids)
    return np.stack([res.results[i]["out"] for i in core_ids], axis=0)


if __name__ == "__main__":
    build_program()
    print("built ok")


# revision 7
# speedup vs baseline: 1.3238x; 1.0327x over previous
"""Trainium2 Bass kernel v2 for the TimeSformer-style divided space-time block.

Data-parallel over B (8 cores). Per core, the residual stream lives in SBUF
feature-major as bf16 for the whole block:

  - x loaded once via DMA-transpose (host pre-casts x to bf16, pads to 1664
    rows); no DRAM round trips between the three stages.
  - LayerNorm is computed feature-major: Sum(x) / Sum(x^2) via ones-matmuls on
    the PE (bf16), ACT Square for x^2, then a 2-pass DVE normalize with
    per-token scale/offset rows partition-broadcast by GpSimd.
  - LN's gamma is folded into the following weight matrix on the host;
    LN's beta contributes b@W which is applied as a per-feature bias at
    eviction (q,k) or through the V columns (softmax rows sum to 1).
  - Branch outputs are accumulated into the residual directly from PSUM with
    fused scalar_tensor_tensor evictions (one rounding per residual add).
  - Spatial attention runs on a frame-major copy of the normalized stream
    (strided per-frame normalize); temporal attention runs in token order
    with the S^T block-diagonal mask trick.
"""

import numpy as np
import ml_dtypes

import concourse.bass as bass
import concourse.mybir as mybir
import concourse.tile as tile
from concourse import bacc

F32 = mybir.dt.float32
BF16 = mybir.dt.bfloat16
AF = mybir.ActivationFunctionType
ALU = mybir.AluOpType
AX = mybir.AxisListType

D = 768
KT = 6
NH = 12
HD = 64
HID = 3072
B = 8
T = 8
HWn = 196
N = 1569
NPAD = 1664
NT = 1568
NF = 197
NS = T * NF
SCALE = HD ** -0.5
P = 128
EPS = 1e-5
INV_D = 1.0 / D

CH_T = [(0, 128), (128, 512), (640, 512), (1152, 416)]
CH_M = [(0, 128), (128, 512), (640, 512), (1152, 417)]
CH_QS = [(0, 1), (1, 1), (2, 2), (4, 2), (6, 2)]

BC_QKT = 0
BC_PRT = 12
BC_TFC = 18
BC_QKS = 24
BC_PRS = 36
BC_FC1 = 42
BC_FC2 = 66

VC_MASK = 0
VC_ID = 128
VC_VBT = 256
VC_VBS = 1024


def tiles_of(n, step=128):
    return [(i, min(step, n - i)) for i in range(0, n, step)]


def build_program(loop_n=0, sim_gelu=False):
    nc = bacc.Bacc("TRN2", target_bir_lowering=False, debug=False, num_devices=8)

    xbf = nc.dram_tensor("xbf", [NPAD, D], BF16, kind="ExternalInput").ap()
    wqkv_t_d = nc.dram_tensor("wqkv_t", [D, 3 * D], BF16, kind="ExternalInput").ap()
    wpr_t_d = nc.dram_tensor("wpr_t", [D, D], BF16, kind="ExternalInput").ap()
    wtfc_d = nc.dram_tensor("wtfc", [D, D], BF16, kind="ExternalInput").ap()
    wqkv_s_d = nc.dram_tensor("wqkv_s", [D, 3 * D], BF16, kind="ExternalInput").ap()
    wpr_s_d = nc.dram_tensor("wpr_s", [D, D], BF16, kind="ExternalInput").ap()
    w1_d = nc.dram_tensor("w1", [D, HID], BF16, kind="ExternalInput").ap()
    w2_d = nc.dram_tensor("w2", [HID, D], BF16, kind="ExternalInput").ap()
    biases_d = nc.dram_tensor("biases", [P, 72], F32, kind="ExternalInput").ap()
    vconst_d = nc.dram_tensor("vconst", [P, 1792], BF16, kind="ExternalInput").ap()
    out = nc.dram_tensor("out", [N, D], F32, kind="ExternalOutput").ap()

    from contextlib import nullcontext

    with tile.TileContext(nc) as tc:
      with tc.tile_pool(name="const", bufs=1) as const:
        eps_sb = const.tile([P, 1], F32, tag="eps")
        nc.vector.memset(eps_sb[:], EPS)
        ones_sb = const.tile([P, 8], BF16, tag="ones")
        nc.vector.memset(ones_sb[:], 1.0)
        oneD_sb = const.tile([P, 1], BF16, tag="oneD")
        nc.vector.memset(oneD_sb[:], INV_D)

        loop_cm = tc.For_i(0, loop_n, 1) if loop_n else nullcontext()
        with loop_cm:
          with tc.tile_pool(name="glob", bufs=1) as glob:
            xT = [glob.tile([P, NPAD], BF16, tag=f"xT{k}", name=f"xT{k}")
                  for k in range(KT)]
            for k in range(KT):
                nc.sync.dma_start(out=xT[k][:], in_=xbf[:, k * P:(k + 1) * P],
                                  transpose=True)
            Bt = glob.tile([P, 72], F32, tag="biases", name="biases")
            nc.sync.dma_start(out=Bt[:], in_=biases_d)
            vc = glob.tile([P, 1792], BF16, tag="vconst", name="vconst")
            nc.sync.dma_start(out=vc[:], in_=vconst_d)
            mask = vc[:, VC_MASK:VC_MASK + P]
            idb = vc[:, VC_ID:VC_ID + P]

            wq = [glob.tile([P, 3 * D], BF16, tag=f"wq{k}", name=f"wq{k}")
                  for k in range(KT)]
            wp = [glob.tile([P, D], BF16, tag=f"wp{k}", name=f"wp{k}")
                  for k in range(KT)]
            for k in range(KT):
                nc.sync.dma_start(out=wq[k][:], in_=wqkv_t_d[k * P:(k + 1) * P, :])
            for k in range(KT):
                nc.sync.dma_start(out=wp[k][:], in_=wpr_t_d[k * P:(k + 1) * P, :])

            cls_save = glob.tile([P, KT], F32, tag="cls", name="cls_save")
            for k in range(KT):
                nc.vector.tensor_copy(cls_save[:, k:k + 1], xT[k][:, 1:2])

            def ln_stats(pool, ps_pool, src_col0, c0, pc, a_bc, c_bc):
                """Per-token scale/offset rows for xT cols [src_col0+c0, +pc)."""
                # ones vector pre-scaled by 1/D: psA = mean, psB = E[x^2]
                psA = ps_pool.tile([P, 512], F32, tag="mm", name="sx")
                for k in range(KT):
                    src = xT[k][:, src_col0 + c0:src_col0 + c0 + pc]
                    nc.tensor.matmul(psA[0:1, :pc], oneD_sb[:, 0:1], src,
                                     start=(k == 0), stop=(k == KT - 1))
                psB = ps_pool.tile([P, 512], F32, tag="mm", name="sq")
                for k in range(KT):
                    src = xT[k][:, src_col0 + c0:src_col0 + c0 + pc]
                    sq = pool.tile([P, 512], BF16, tag="sqv", name="sqv", bufs=2)
                    nc.scalar.activation(sq[:, :pc], src, AF.Square)
                    nc.tensor.matmul(psB[0:1, :pc], oneD_sb[:, 0:1], sq[:, :pc],
                                     start=(k == 0), stop=(k == KT - 1))
                mu = pool.tile([1, 512], F32, tag="mu", name="mu", bufs=2)
                nc.vector.tensor_copy(mu[:, :pc], psA[0:1, :pc])
                r2 = pool.tile([1, 512], F32, tag="r2", name="r2", bufs=2)
                # var = E[x^2] - mu^2
                nc.vector.scalar_tensor_tensor(r2[:, :pc], mu[:, :pc], -1.0,
                                               mu[:, :pc], ALU.mult, ALU.mult)
                nc.vector.tensor_tensor(r2[:, :pc], psB[0:1, :pc], r2[:, :pc],
                                        ALU.add)
                # inv = exp(-0.5*ln(var+eps)); Ln and Exp live in the same ACT
                # table as Square/Identity (natural_log_exp_and_others)
                nc.scalar.activation(r2[:, :pc], r2[:, :pc], AF.Ln,
                                     bias=eps_sb[0:1])
                a_row = pool.tile([1, 512], BF16, tag="arow", name="arow", bufs=2)
                nc.scalar.activation(a_row[:, :pc], r2[:, :pc], AF.Exp, scale=-0.5)
                c_row = pool.tile([1, 512], BF16, tag="crow", name="crow", bufs=2)
                nc.vector.tensor_tensor(c_row[:, :pc], mu[:, :pc],
                                        a_row[:, :pc], ALU.mult)
                nc.gpsimd.partition_broadcast(a_bc[:, c0:c0 + pc], a_row[0:1, :pc])
                nc.gpsimd.partition_broadcast(c_bc[:, c0:c0 + pc], c_row[0:1, :pc])

            # =====================================================
            # Stage T
            # =====================================================
            with tc.tile_pool(name="t_sb", bufs=1) as sbT, \
                 tc.tile_pool(name="t_work", bufs=3) as work, \
                 tc.tile_pool(name="t_mm", bufs=3, space="PSUM") as pmm, \
                 tc.tile_pool(name="t_att", bufs=3, space="PSUM") as patt, \
                 tc.tile_pool(name="t_tr", bufs=2, space="PSUM") as ptr:

                wtfc = [sbT.tile([P, D], BF16, tag=f"wt{k}", name=f"wt{k}")
                        for k in range(KT)]
                for k in range(KT):
                    nc.sync.dma_start(out=wtfc[k][:], in_=wtfc_d[k * P:(k + 1) * P, :])

                aT = sbT.tile([P, NT], BF16, tag="aT", name="aT")
                cT = sbT.tile([P, NT], BF16, tag="cT", name="cT")
                lnT = [sbT.tile([P, NT], BF16, tag=f"ln{k}", name=f"lnT{k}")
                       for k in range(KT)]
                qkT = [sbT.tile([P, NT], BF16, tag=f"qk{j}", name=f"qkT{j}")
                       for j in range(12)]
                v_t = [sbT.tile([P, NH, HD + 1], BF16, tag=f"vt{g}", name=f"vt{g}")
                       for g in range(13)]
                oT = [sbT.tile([P, NT], BF16, tag=f"oT{k}", name=f"oTt{k}")
                      for k in range(KT)]

                for (c0, pc) in CH_T:
                    ln_stats(work, pmm, 1, c0, pc, aT, cT)
                    for k in range(KT):
                        tmp = work.tile([P, 512], BF16, tag="nt", name="nt", bufs=3)
                        nc.vector.tensor_tensor(tmp[:, :pc],
                                                xT[k][:, 1 + c0:1 + c0 + pc],
                                                aT[:, c0:c0 + pc], ALU.mult)
                        nc.vector.tensor_tensor(lnT[k][:, c0:c0 + pc], tmp[:, :pc],
                                                cT[:, c0:c0 + pc], ALU.subtract)
                    for mi in range(12):
                        ps = pmm.tile([P, 512], F32, tag="mm", name="mm")
                        for k in range(KT):
                            nc.tensor.matmul(ps[:, :pc],
                                             wq[k][:, mi * P:(mi + 1) * P],
                                             lnT[k][:, c0:c0 + pc],
                                             start=(k == 0), stop=(k == KT - 1))
                        nc.scalar.activation(qkT[mi][:, c0:c0 + pc], ps[:, :pc],
                                             AF.Identity, bias=Bt[:, BC_QKT + mi:BC_QKT + mi + 1])
                    for (g0, gp) in tiles_of(pc):
                        g = (c0 + g0) // P
                        t0 = c0 + g0
                        nc.vector.memset(v_t[g][:gp, :, HD:HD + 1], 1.0)
                        for half in range(2):
                            ps = pmm.tile([P, 512], F32, tag="mm", name="mmv")
                            for k in range(KT):
                                nc.tensor.matmul(
                                    ps[:gp, :384],
                                    lnT[k][:, t0:t0 + gp],
                                    wq[k][:, 2 * D + 384 * half:2 * D + 384 * (half + 1)],
                                    start=(k == 0), stop=(k == KT - 1))
                            nc.vector.scalar_tensor_tensor(
                                v_t[g][:gp, 6 * half:6 * (half + 1), 0:HD],
                                ps[:gp, :384].rearrange("p (a c) -> p a c", a=6),
                                1.0,
                                vc[0:gp, VC_VBT + 384 * half:VC_VBT + 384 * (half + 1)]
                                .rearrange("p (a c) -> p a c", a=6),
                                ALU.mult, ALU.add)
                    for (g0, gp) in tiles_of(pc):
                        g = (c0 + g0) // P
                        t0 = c0 + g0
                        o_tm = work.tile([P, D], BF16, tag="otm", name="otm", bufs=2)
                        for h in range(NH):
                            j, r0 = h // 2, HD * (h % 2)
                            att = patt.tile([P, P + HD + 1], F32, tag="att",
                                            name="att")
                            st = att[:, 0:P]
                            ov = att[:, P:P + HD + 1]
                            nc.tensor.matmul(st[:gp, :gp],
                                             qkT[6 + j][r0:r0 + HD, t0:t0 + gp],
                                             qkT[j][r0:r0 + HD, t0:t0 + gp],
                                             start=True, stop=True)
                            es = work.tile([P, P], BF16, tag="es", name="es", bufs=4)
                            nc.scalar.activation(es[:gp, :gp], st[:gp, :gp], AF.Exp,
                                                 scale=SCALE)
                            nc.vector.tensor_tensor(es[:gp, :gp], es[:gp, :gp],
                                                    mask[0:gp, 0:gp], ALU.mult)
                            nc.tensor.matmul(ov[:gp, :], es[:gp, :gp],
                                             v_t[g][:gp, h, :], start=True, stop=True)
                            rec = work.tile([P, 1], F32, tag="rec", name="rec", bufs=4)
                            nc.vector.reciprocal(rec[:gp], ov[:gp, HD:HD + 1])
                            nc.vector.tensor_scalar_mul(o_tm[:gp, HD * h:HD * (h + 1)],
                                                        ov[:gp, 0:HD], rec[:gp])
                        for k in range(KT):
                            ps = ptr.tile([P, P], BF16, tag="tr", name="tr")
                            nc.tensor.transpose(ps[:, :gp], o_tm[:gp, k * P:(k + 1) * P],
                                                idb[0:gp, 0:gp])
                            nc.vector.tensor_copy(oT[k][:, t0:t0 + gp],
                                                  ps[:, :gp])




                # proj -> pT (chunk-local) -> tfc -> residual accumulate
                for (c0, pc) in CH_T:
                    pTc = []
                    for mi in range(KT):
                        ps = pmm.tile([P, 512], F32, tag="mm", name="mmp")
                        for k in range(KT):
                            nc.tensor.matmul(ps[:, :pc],
                                             wp[k][:, mi * P:(mi + 1) * P],
                                             oT[k][:, c0:c0 + pc],
                                             start=(k == 0), stop=(k == KT - 1))
                        pT = work.tile([P, 512], BF16, tag=f"pT{mi}",
                                       name=f"pT{mi}", bufs=2)
                        nc.scalar.activation(pT[:, :pc], ps[:, :pc],
                                             AF.Identity, bias=Bt[:, BC_PRT + mi:BC_PRT + mi + 1])
                        pTc.append(pT)
                    for mi in range(KT):
                        ps = pmm.tile([P, 512], F32, tag="mm", name="mmt")
                        for k in range(KT):
                            nc.tensor.matmul(ps[:, :pc],
                                             wtfc[k][:, mi * P:(mi + 1) * P],
                                             pTc[k][:, :pc],
                                             start=(k == 0), stop=(k == KT - 1))
                        nc.vector.scalar_tensor_tensor(
                            xT[mi][:, 1 + c0:1 + c0 + pc], ps[:, :pc],
                            Bt[:, BC_TFC + mi:BC_TFC + mi + 1],
                            xT[mi][:, 1 + c0:1 + c0 + pc], ALU.add, ALU.add)

                # spatial weights go into the same tiles, after the last
                # temporal reads (program order guarantees correctness)
                for k in range(KT):
                    nc.sync.dma_start(out=wq[k][:], in_=wqkv_s_d[k * P:(k + 1) * P, :])
                for k in range(KT):
                    nc.sync.dma_start(out=wp[k][:], in_=wpr_s_d[k * P:(k + 1) * P, :])

            # =====================================================
            # Stage S
            # =====================================================
            with tc.tile_pool(name="s_sb", bufs=1) as sbS, \
                 tc.tile_pool(name="s_work", bufs=3) as work, \
                 tc.tile_pool(name="s_mm", bufs=3, space="PSUM") as pmm, \
                 tc.tile_pool(name="s_st", bufs=2, space="PSUM") as pst, \
                 tc.tile_pool(name="s_ov", bufs=2, space="PSUM") as pov:

                aS = sbS.tile([P, NT], BF16, tag="aS", name="aS")
                cS = sbS.tile([P, NT], BF16, tag="cS", name="cS")
                lnS = [sbS.tile([P, NS], BF16, tag=f"lnS{k}", name=f"lnS{k}")
                       for k in range(KT)]
                qkS = [sbS.tile([P, NS], BF16, tag=f"qkS{j}", name=f"qkS{j}")
                       for j in range(12)]
                oS = [sbS.tile([P, NS], BF16, tag=f"oS{k}", name=f"oSs{k}")
                      for k in range(KT)]

                lnStok = [sbS.tile([P, NT], BF16, tag=f"lnK{k}", name=f"lnStok{k}")
                          for k in range(KT)]
                for (c0, pc) in CH_T:
                    ln_stats(work, pmm, 1, c0, pc, aS, cS)
                    for k in range(KT):
                        tmp = work.tile([P, 512], BF16, tag="ns", name="ns", bufs=3)
                        nc.vector.tensor_tensor(tmp[:, :pc],
                                                xT[k][:, 1 + c0:1 + c0 + pc],
                                                aS[:, c0:c0 + pc], ALU.mult)
                        nc.vector.tensor_tensor(lnStok[k][:, c0:c0 + pc],
                                                tmp[:, :pc],
                                                cS[:, c0:c0 + pc], ALU.subtract)

                # cls token LN (from the saved original x[1])
                cls_bf = work.tile([P, KT], BF16, tag="clsbf", name="cls_bf")
                nc.vector.tensor_copy(cls_bf[:], cls_save[:])
                psc1 = pmm.tile([P, 512], F32, tag="mm", name="clsx")
                for k in range(KT):
                    nc.tensor.matmul(psc1[0:1, 0:1], ones_sb[:, 0:1],
                                     cls_bf[:, k:k + 1],
                                     start=(k == 0), stop=(k == KT - 1))
                sqc = work.tile([P, KT], BF16, tag="sqc", name="sqc")
                nc.scalar.activation(sqc[:], cls_save[:], AF.Square)
                psc2 = pmm.tile([P, 512], F32, tag="mm", name="clsq")
                for k in range(KT):
                    nc.tensor.matmul(psc2[0:1, 0:1], ones_sb[:, 0:1], sqc[:, k:k + 1],
                                     start=(k == 0), stop=(k == KT - 1))
                muc = work.tile([1, 2], F32, tag="muc", name="muc")
                nc.vector.tensor_scalar_mul(muc[:, 0:1], psc1[0:1, 0:1], INV_D)
                varc = work.tile([1, 1], F32, tag="varc", name="varc")
                nc.vector.scalar_tensor_tensor(varc[:], muc[:, 0:1], -1.0,
                                               muc[:, 0:1], ALU.mult, ALU.mult)
                nc.vector.scalar_tensor_tensor(varc[:], psc2[0:1, 0:1], INV_D,
                                               varc[:], ALU.mult, ALU.add)
                invc = work.tile([1, 1], F32, tag="invc", name="invc")
                nc.scalar.activation(invc[:], varc[:], AF.Ln, bias=eps_sb[0:1])
                nc.scalar.activation(invc[:], invc[:], AF.Exp, scale=-0.5)
                stc = work.tile([P, 2], F32, tag="stc", name="stc")
                nc.gpsimd.partition_broadcast(stc[:, 0:1], muc[:, 0:1])
                nc.gpsimd.partition_broadcast(stc[:, 1:2], invc[:, 0:1])
                lncls = work.tile([P, KT], F32, tag="lncls", name="lncls")
                nc.vector.tensor_scalar(lncls[:], cls_save[:], stc[:, 0:1],
                                        stc[:, 1:2], ALU.subtract, ALU.mult)
                for k in range(KT):
                    nc.vector.tensor_scalar_mul(
                        lnS[k].rearrange("p (t n) -> p t n", t=T)[:, :, 0:1],
                        ones_sb[:, 0:8].rearrange("p (t n) -> p t n", t=T),
                        lncls[:, k:k + 1])

                def fview(ap1568, f):
                    return (ap1568.rearrange("p (w t) -> p t w", t=T)
                            [:, f:f + 1, :].rearrange("p a w -> p (a w)"))

                # scatter token-order lnStok into frame-major lnS, split
                # across ACT and DVE
                for f in range(T):
                    for k in range(KT):
                        src = fview(lnStok[k][:], f)
                        dst = lnS[k][:, f * NF + 1:(f + 1) * NF]
                        if (f * KT + k) % 2 == 0:
                            nc.scalar.activation(dst, src, AF.Copy)
                        else:
                            nc.vector.tensor_copy(dst, src)

                for (f0, nf) in CH_QS:
                    c0, pc = f0 * NF, nf * NF
                    for mi in range(12):
                        ps = pmm.tile([P, 512], F32, tag="mm", name="mmqs")
                        for k in range(KT):
                            nc.tensor.matmul(ps[:, :pc],
                                             wq[k][:, mi * P:(mi + 1) * P],
                                             lnS[k][:, c0:c0 + pc],
                                             start=(k == 0), stop=(k == KT - 1))
                        nc.scalar.activation(qkS[mi][:, c0:c0 + pc], ps[:, :pc],
                                             AF.Identity, bias=Bt[:, BC_QKS + mi:BC_QKS + mi + 1])
                    v_s = {}
                    for f in range(f0, f0 + nf):
                        for i, (k0, pk) in enumerate(tiles_of(NF)):
                            v_s.setdefault(f, {})[i] = sbS.tile(
                                [P, NH, HD + 1], BF16,
                                tag=f"vs{(f % 2) * 2 + i}",
                                name=f"vs{(f % 2) * 2 + i}", bufs=2)
                            nc.vector.memset(v_s[f][i][:pk, :, HD:HD + 1], 1.0)
                            for half in range(2):
                                ps = pmm.tile([P, 512], F32, tag="mm", name="mmvs")
                                for k in range(KT):
                                    nc.tensor.matmul(
                                        ps[:pk, :384],
                                        lnS[k][:, f * NF + k0:f * NF + k0 + pk],
                                        wq[k][:, 2 * D + 384 * half:2 * D + 384 * (half + 1)],
                                        start=(k == 0), stop=(k == KT - 1))
                                nc.vector.scalar_tensor_tensor(
                                    v_s[f][i][:pk, 6 * half:6 * (half + 1), 0:HD],
                                    ps[:pk, :384].rearrange("p (a c) -> p a c", a=6),
                                    1.0,
                                    vc[0:pk, VC_VBS + 384 * half:VC_VBS + 384 * (half + 1)]
                                    .rearrange("p (a c) -> p a c", a=6),
                                    ALU.mult, ALU.add)
                    for f in range(f0, f0 + nf):
                        fc = f * NF
                        for h in range(NH):
                            j, r0 = h // 2, HD * (h % 2)
                            qs = qkS[j][r0:r0 + HD, fc:fc + NF]
                            es_list = []
                            for i, (k0, pk) in enumerate(tiles_of(NF)):
                                st = pst.tile([P, NF], F32, tag="st", name="stS")
                                nc.tensor.matmul(st[:pk, :NF],
                                                 qkS[6 + j][r0:r0 + HD,
                                                            fc + k0:fc + k0 + pk],
                                                 qs, start=True, stop=True)
                                es = work.tile([P, NF], BF16, tag="esS", name="esS",
                                               bufs=4)
                                nc.scalar.activation(es[:pk, :NF], st[:pk, :NF],
                                                     AF.Exp, scale=SCALE)
                                es_list.append((es, k0, pk))
                            ov = pov.tile([HD + 1, NF], F32, tag="ov", name="ovS")
                            for i, (es, k0, pk) in enumerate(es_list):
                                nc.tensor.matmul(ov[:, :NF], v_s[f][i][:pk, h, :],
                                                 es[:pk, :NF], start=(i == 0),
                                                 stop=(i == len(es_list) - 1))
                            rec = work.tile([1, NF], F32, tag="recS", name="recS",
                                            bufs=4)
                            nc.vector.reciprocal(rec[:1, :], ov[HD:HD + 1, :])
                            bc = work.tile([HD, NF], F32, tag="bcS", name="bcS",
                                           bufs=4)
                            nc.gpsimd.partition_broadcast(bc[:, :], rec[0:1, :])
                            nc.vector.tensor_tensor(oS[j][r0:r0 + HD, fc:fc + NF],
                                                    ov[0:HD, :NF], bc[:, :], ALU.mult)


                for (f0, nf) in CH_QS:
                    c0, pc = f0 * NF, nf * NF
                    for mi in range(KT):
                        ps = pmm.tile([P, 512], F32, tag="mm", name="mmps")
                        for k in range(KT):
                            nc.tensor.matmul(ps[:, :pc],
                                             wp[k][:, mi * P:(mi + 1) * P],
                                             oS[k][:, c0:c0 + pc],
                                             start=(k == 0), stop=(k == KT - 1))
                        for f in range(f0, f0 + nf):
                            off = (f - f0) * NF
                            nc.vector.scalar_tensor_tensor(
                                fview(xT[mi][:, 1:1 + NT], f),
                                ps[:, off + 1:off + NF],
                                Bt[:, BC_PRS + mi:BC_PRS + mi + 1],
                                fview(xT[mi][:, 1:1 + NT], f), ALU.add, ALU.add)

                # cls_out = proj(mean over frames of attention-out cls cols)
                oTc = work.tile([P, KT], BF16, tag="oTc", name="oTc")
                for k in range(KT):
                    red = work.tile([P, 1], F32, tag="redc", name="redc", bufs=2)
                    nc.vector.tensor_reduce(
                        red[:],
                        oS[k].rearrange("p (t n) -> p n t", t=T)[:, 0:1, :],
                        AX.X, ALU.add)
                    nc.vector.tensor_scalar_mul(oTc[:, k:k + 1], red[:], 1.0 / T)
                for mi in range(KT):
                    psc = pmm.tile([P, 512], F32, tag="mm", name="clsp")
                    for k in range(KT):
                        nc.tensor.matmul(psc[:, 0:1], wp[k][:, mi * P:(mi + 1) * P],
                                         oTc[:, k:k + 1],
                                         start=(k == 0), stop=(k == KT - 1))
                    nc.vector.scalar_tensor_tensor(
                        xT[mi][:, 0:1], psc[:, 0:1], Bt[:, BC_PRS + mi:BC_PRS + mi + 1],
                        cls_save[:, mi:mi + 1], ALU.add, ALU.add)

                # W2 into the soon-free wq/wp tiles (emitted after last reads)
                w2v = []
                for j in range(24):
                    if j < 18:
                        tgt = wq[j // 3][:, D * (j % 3):D * (j % 3 + 1)]
                    else:
                        tgt = wp[j - 18][:]
                    nc.sync.dma_start(out=tgt, in_=w2_d[j * P:(j + 1) * P, :])
                    w2v.append(tgt)

            # =====================================================
            # Stage M (MLP)
            # =====================================================
            with tc.tile_pool(name="m_sb", bufs=1) as sbM, \
                 tc.tile_pool(name="m_g", bufs=2) as gpool, \
                 tc.tile_pool(name="m_work", bufs=3) as work, \
                 tc.tile_pool(name="m_mm", bufs=3, space="PSUM") as pmm, \
                 tc.tile_pool(name="m_tr", bufs=3, space="PSUM") as ptr:

                aM = sbM.tile([P, N], BF16, tag="aM", name="aM")
                cM = sbM.tile([P, N], BF16, tag="cM", name="cM")
                lnM = [sbM.tile([P, N], BF16, tag=f"lnM{k}", name=f"lnM{k}")
                       for k in range(KT)]
                w1 = [sbM.tile([P, HID], BF16, tag=f"w1{k}", name=f"w1{k}")
                      for k in range(KT)]
                for k in range(KT):
                    nc.sync.dma_start(out=w1[k][:], in_=w1_d[k * P:(k + 1) * P, :])

                # all stats + normalizes first so the ACT table switches
                # ln_exp -> gelu exactly once per iteration
                for (c0, pc) in CH_M:
                    ln_stats(work, pmm, 0, c0, pc, aM, cM)
                    for k in range(KT):
                        tmp = work.tile([P, 512], BF16, tag="nt", name="ntm", bufs=3)
                        nc.vector.tensor_tensor(tmp[:, :pc], xT[k][:, c0:c0 + pc],
                                                aM[:, c0:c0 + pc], ALU.mult)
                        nc.vector.tensor_tensor(lnM[k][:, c0:c0 + pc], tmp[:, :pc],
                                                cM[:, c0:c0 + pc], ALU.subtract)
                for (c0, pc) in CH_M:
                    g1T = [gpool.tile([P, 512], BF16, tag=f"g1T{m}", name=f"g1T{m}")
                           for m in range(24)]
                    for m in range(24):
                        ps = pmm.tile([P, 512], F32, tag="mm", name="f1ps")
                        for k in range(KT):
                            nc.tensor.matmul(ps[:, :pc], w1[k][:, m * P:(m + 1) * P],
                                             lnM[k][:, c0:c0 + pc],
                                             start=(k == 0), stop=(k == KT - 1))
                        if sim_gelu:
                            hb = work.tile([P, 512], F32, tag="hb", name="hb", bufs=1)
                            nc.scalar.activation(hb[:, :pc], ps[:, :pc], AF.Identity,
                                                 bias=Bt[:, BC_FC1 + m:BC_FC1 + m + 1])
                            sg = work.tile([P, 512], F32, tag="sg", name="sg", bufs=1)
                            nc.scalar.activation(sg[:, :pc], hb[:, :pc], AF.Sigmoid,
                                                 scale=1.702)
                            nc.vector.tensor_tensor(g1T[m][:, :pc], hb[:, :pc],
                                                    sg[:, :pc], ALU.mult)
                        else:
                            nc.scalar.activation(g1T[m][:, :pc], ps[:, :pc], AF.Gelu,
                                                 bias=Bt[:, BC_FC1 + m:BC_FC1 + m + 1])
                    for mi in range(KT):
                        ps = pmm.tile([P, 512], F32, tag="mm", name="f2ps")
                        for k in range(24):
                            nc.tensor.matmul(ps[:, :pc], w2v[k][:, mi * P:(mi + 1) * P],
                                             g1T[k][:, :pc],
                                             start=(k == 0), stop=(k == 23))
                        nc.vector.scalar_tensor_tensor(
                            xT[mi][:, c0:c0 + pc], ps[:, :pc], Bt[:, BC_FC2 + mi:BC_FC2 + mi + 1],
                            xT[mi][:, c0:c0 + pc], ALU.add, ALU.add)
                    for (q0, pq) in tiles_of(pc):
                        t0 = c0 + q0
                        out_sb = work.tile([P, D], F32, tag="osb", name="osb", bufs=3)
                        for k in range(KT):
                            ps = ptr.tile([P, P], BF16, tag="tr", name="otr")
                            nc.tensor.transpose(ps[:pq, :], xT[k][:, t0:t0 + pq],
                                                idb[:, :])
                            nc.vector.tensor_copy(out_sb[:pq, k * P:(k + 1) * P],
                                                  ps[:pq, :])
                        nc.sync.dma_start(out=out[t0:t0 + pq, :], in_=out_sb[:pq])

    nc.compile()
    return nc


_CACHED = {}


def _get_program():
    if "nc" not in _CACHED:
        _CACHED["nc"] = build_program()
    return _CACHED["nc"]


def _host_prep(inputs):
    f32 = np.float32
    g = lambda k: np.asarray(inputs[k], f32)
    x = g("x")
    gt, bt = g("gt"), g("bt")
    g1, b1 = g("g1"), g("b1")
    g2, b2 = g("g2"), g("b2")
    Wqkv_t, Wproj_t, bproj_t = g("Wqkv_t"), g("Wproj_t"), g("bproj_t")
    Wqkv_s, Wproj_s, bproj_s = g("Wqkv_s"), g("Wproj_s"), g("bproj_s")
    Wtfc, btfc = g("Wtfc"), g("btfc")
    W1, b1m = g("W1"), g("b1m")
    W2, b2m = g("W2"), g("b2m")

    bf = ml_dtypes.bfloat16
    wqkv_t = np.ascontiguousarray((gt[:, None] * Wqkv_t).astype(bf))
    wqkv_s = np.ascontiguousarray((g1[:, None] * Wqkv_s).astype(bf))
    w1 = np.ascontiguousarray((g2[:, None] * W1).astype(bf))
    qkvb_t = bt @ Wqkv_t
    qkvb_s = b1 @ Wqkv_s
    b1m_f = b2 @ W1 + b1m

    def cols(vec, n):
        return np.asarray(vec, f32).reshape(n, P).T

    biases = np.concatenate([
        cols(qkvb_t[:2 * D], 12), cols(bproj_t, 6), cols(btfc, 6),
        cols(qkvb_s[:2 * D], 12), cols(bproj_s, 6),
        cols(b1m_f, 24), cols(b2m, 6)], axis=1).astype(f32)

    mask = np.kron(np.eye(16, dtype=f32), np.ones((8, 8), f32))
    ident = np.eye(P, dtype=f32)
    vb_t = np.tile(qkvb_t[2 * D:], (P, 1))
    vb_s = np.tile(qkvb_s[2 * D:], (P, 1))
    vconst = np.concatenate([mask, ident, vb_t, vb_s], axis=1).astype(bf)

    base = {
        "wqkv_t": wqkv_t, "wpr_t": np.ascontiguousarray(Wproj_t.astype(bf)),
        "wtfc": np.ascontiguousarray(Wtfc.astype(bf)),
        "wqkv_s": wqkv_s, "wpr_s": np.ascontiguousarray(Wproj_s.astype(bf)),
        "w1": w1, "w2": np.ascontiguousarray(W2.astype(bf)),
        "biases": np.ascontiguousarray(biases),
        "vconst": np.ascontiguousarray(vconst),
    }
    maps = []
    for i in range(B):
        xb = np.zeros((NPAD, D), bf)
        xb[:N] = x[i].astype(bf)
        maps.append(dict(base, xbf=np.ascontiguousarray(xb)))
    return maps


def make_in_maps(inputs):
    return _host_prep(inputs)


def kernel(**inputs):
    nc = _get_program()
    in_maps = make_in_maps(inputs)
    core_ids = list(range(8))
    from concourse.bass_utils import run_bass_kernel_spmd
    res = run_bass_kernel_spmd(nc, in_maps, core_ids)
    return np.stack([res.results[i]["out"] for i in core_ids], axis=0)


if __name__ == "__main__":
    build_program()
    print("built ok")
